# revision 41
# baseline (speedup 1.0000x reference)
"""AtlasFreeBrainTransformer Trainium2 kernel (fp8 DoubleRow version).

Host contract: kernel(**inputs) takes the FULL unsharded inputs of
reference.setup_inputs() and returns the FULL (B, 2) float32 output.

Sharding: data-parallel over batch B=8 across the 8 NeuronCores (one
batch element per core, weights replicated, no collectives).  The
valid-node mask reduces over batch; it is computed on the host directly
from the integer index tensor C, so no cross-core reduction is needed.

The gather + reduce_window is collapsed into one matmul with a host-built
integer count matrix S (see _build_counts).

Device pipeline per core (all-transposed layout, x_T is (EMB, T)):
  embed FFN (fp32r, exact) -> nodes via S contraction (fp32r, exact)
  -> DEPTH transformer layers where every big matmul runs in fp8e4m3,
     most in DoubleRow perf mode (2x128 contraction per pass at 0.5
     cyc/row):
       qkv (DR), scores (fp8), exp with constant shift so the softmax
       numerator/denominator fit fp8, AV with key-tile pairs (DR),
       per-head normalize fused into the fp8 quantize copy, out-proj
       with head pairs (DR), FFN1/FFN2 (DR).
     Residual stream stays fp32r.  LayerNorm stats via ones-matmuls +
     DRAM-roundtrip reshape + rsqrt bit trick (as before).
  -> mean over tokens -> 3-layer classifier head (fp32) -> logits.

Elementwise work is split DVE/Pool(gpsimd) to keep the ACT engine free
for exp/gelu (the bottleneck).
"""

import sys

sys.path.insert(0, "/opt/trn_rl_repo")

import math
from contextlib import ExitStack

import numpy as np
import ml_dtypes

import concourse.bass as bass
import concourse.tile as tile
from concourse import bacc, mybir
from concourse.bass_utils import run_bass_kernel_spmd

F32 = mybir.dt.float32
F32R = mybir.dt.float32r
BF16 = mybir.dt.bfloat16
F8 = mybir.dt.float8e4
AF = mybir.ActivationFunctionType
ALU = mybir.AluOpType
AX = mybir.AxisListType
DRM = mybir.MatmulPerfMode.DoubleRow
NPF8 = ml_dtypes.float8_e4m3

# Model dims (hardcoded per problem spec)
B, NROI, DF, G, EMB, NH, HD, FFD, DEPTH = 8, 400, 512, 25, 360, 4, 90, 2048, 2
KS, ST = 3, 2
NBLK = (G - KS) // ST + 1          # 12
NB = NBLK ** 3                     # 1728 nodes
EPS = 1e-5
H450 = 450
C1, C2, NCLS = 256, 128, 2
HD1 = HD + 1                       # 91 (ones row for the denominator)
HD2 = HD + 2                       # 92: even block so DoubleRow lowers
EMBP = 368   # EMB padded so DoubleRow weight strides stay 16B-aligned

QCH = 448    # fp32 psum chunk that fits one 2KB bank
QH = 896     # attention / LN q chunk (2 banks)
PCH = 512    # matmul sub-chunk inside a 2-bank psum tile (bank boundary)
EXPS = -1.5  # exp(score*scale + EXPS): keeps softmax terms in fp8 range
OSCL = 64.0  # osbN = O * OSCL / den so fp8/bf16 sees ~unit-scale values
SMPL = 448   # token-sample count for the rank-1 mean corrections
DSC = 256.0  # fp8 residual-weight upscale (undone on device)
CSC = 1.0 / (DSC * SMPL)

# EMB row chunks: A,B are the DoubleRow contraction pair, C is the tail
XCH = [(0, 128), (128, 128), (256, 104)]


def chunks(total, size):
    out = []
    s = 0
    while s < total:
        out.append((s, min(size, total - s)))
        s += size
    return out


class Builder:
    def __init__(self, nc, tc, ctx, T, Tpad, flags, dbg=False):
        self.nc = nc
        self.tc = tc
        self.ctx = ctx
        self.T = T
        self.Tpad = Tpad
        self.flags = flags
        self.dbg = dbg
        self.dram = {}
        # fp8-rounded pad-key softmax term, dequantized: the device AV
        # matmul accumulates exactly this value per zero-padded key
        self.padc = float(
            np.float32(np.exp(np.float32(EXPS))).astype(NPF8).astype(np.float32)
        ) * (Tpad - T)

    # ---------- dram declarations ----------
    def din(self, name, shape, dtype=F32):
        t = self.nc.dram_tensor(name, list(shape), dtype, kind="ExternalInput")
        self.dram[name] = t.ap()
        return self.dram[name]

    def dout(self, name, shape, dtype=F32):
        t = self.nc.dram_tensor(name, list(shape), dtype, kind="ExternalOutput")
        self.dram[name] = t.ap()
        return self.dram[name]

    def debug_dump(self, name, parts):
        if not self.dbg:
            return
        rows = max(s + ap.shape[0] for s, ap in parts)
        cols = parts[0][1].shape[1] if len(parts[0][1].shape) > 1 else 1
        d = self.dout(f"dbg_{name}", (rows, cols), parts[0][1].dtype)
        for s, ap in parts:
            self.nc.sync.dma_start(out=d[s : s + ap.shape[0], :], in_=ap)

    # ---------- small helpers ----------
    def load_rows(self, pool, dram_ap, row_chunks, cols, dtype=F32, name="w",
                  eng=None):
        tiles = []
        eng = eng or self.nc.scalar
        for i, (s, sz) in enumerate(row_chunks):
            t = pool.tile([sz, cols], dtype, name=f"{name}{i}", tag=f"{name}{i}")
            eng.dma_start(out=t, in_=dram_ap[s : s + sz, :])
            tiles.append(t)
        return tiles

    def load_cols(self, pool, dram_ap, row_chunks, dtype=F32, name="c",
                  eng=None):
        return self.load_rows(pool, dram_ap, row_chunks, 1, dtype=dtype,
                              name=name, eng=eng)

    # ---------- build ----------
    def build(self):
        nc, tc, ctx = self.nc, self.tc, self.ctx

        consts = ctx.enter_context(tc.tile_pool(name="consts", bufs=1))
        self._ones_col = self.load_rows(
            consts, self.din("ones_col", (128, 1), F32R), [(0, 128)], 1,
            dtype=F32R, name="ones_col")[0]
        self._ones_row = self.load_rows(
            consts, self.din("ones_row", (1, 128), F32R), [(0, 1)], 128,
            dtype=F32R, name="ones_row")[0]
        self._expb = consts.tile([128, 1], F32, name="expb", tag="expb")
        nc.vector.memset(self._expb, EXPS)
        self._ones_col_b = consts.tile([128, 1], BF16, name="ones_col_b",
                                       tag="ones_col_b")
        nc.vector.memset(self._ones_col_b, 1.0)
        self.dscr = ctx.enter_context(
            tc.tile_pool(name="dscr", bufs=1, space="DRAM"))
        # persistent rows for the pipelined LN apply (a2/b2 survive into the
        # next phase where the deferred apply is emitted)
        self.lnrows = ctx.enter_context(tc.tile_pool(name="lnrows", bufs=2))

        # persistent pools first (pool lifetimes must nest LIFO); the
        # embed pool opens last so it can close right after the embed
        # phase.  Embed weight DMAs are issued before the big transformer
        # weight DMAs so the first phase is never stuck behind them.
        wpool = ctx.enter_context(tc.tile_pool(name="wpool", bufs=1))
        xpool = ctx.enter_context(tc.tile_pool(name="xpool", bufs=2))
        x8pool = ctx.enter_context(tc.tile_pool(name="x8pool", bufs=2))
        self.x8pool = x8pool
        self._estack = ExitStack()
        epool = self._estack.enter_context(tc.tile_pool(name="embed", bufs=1))
        emb = self.load_embed(epool)
        self.load_weights(wpool)

        xt, x8, x8c = self.phase_embed_nodes(xpool, emb)

        pending = None
        for l in range(DEPTH):
            xt, x8, x8c, pending = self.phase_layer(l, xt, x8, x8c, xpool,
                                                    pending)

        self.phase_head(xt, pending)

    def load_embed(self, epool):
        Tpad = self.Tpad
        w1d = self.din("w1", (DF, H450), F32R)
        b1d = self.din("b1c", (H450, 1))
        w2d = self.din("w2", (H450, EMB), F32R)
        b2r = self.din("b2r", (1, EMB), F32R)
        frd = self.din("f_roiT", (DF, NROI), F32R)
        std = self.din("s_t", (NROI, Tpad), F32R)
        kch_df = chunks(DF, 128)
        mch_450 = chunks(H450, 128)
        mch_400 = chunks(NROI, 128)
        emb = {}
        emb["w1t"] = self.load_rows(epool, w1d, kch_df, H450, dtype=F32R,
                                    name="w1t")
        emb["frt"] = self.load_rows(epool, frd, kch_df, NROI, dtype=F32R,
                                    name="frt", eng=self.nc.sync)
        emb["b1c"] = self.load_cols(epool, b1d, mch_450, name="b1c",
                                    eng=self.nc.sync)
        emb["w2t"] = self.load_rows(epool, w2d, mch_450, EMB, dtype=F32R,
                                    name="w2t", eng=self.nc.sync)
        emb["b2rt"] = self.load_rows(epool, b2r, [(0, 1)], EMB, dtype=F32R,
                                     name="b2rt", eng=self.nc.sync)[0]
        emb["stt"] = self.load_rows(epool, std, mch_400, Tpad, dtype=F32R,
                                    name="stt", eng=self.nc.sync)
        emb["pool"] = epool
        return emb

    def load_weights(self, wpool):
        f = self.flags
        self.W = {}
        for l in range(DEPTH):
            W = {}
            W["wqk8"] = self.load_rows(
                wpool, self.din(f"wqk8d{l}", (128, 1440), F8), [(0, 128)],
                1440, dtype=F8, name=f"wqk8_{l}", eng=self.nc.sync)[0]
            W["wqk8c"] = self.load_rows(
                wpool, self.din(f"wqk8c{l}", (104, 720), F8), [(0, 104)],
                720, dtype=F8, name=f"wqk8c_{l}", eng=self.nc.sync)[0]
            W["wv8"] = self.load_rows(
                wpool, self.din(f"wv8d{l}", (128, 2 * EMBP), F8), [(0, 128)],
                2 * EMBP, dtype=F8, name=f"wv8_{l}", eng=self.nc.sync)[0]
            W["wv8c"] = self.load_rows(
                wpool, self.din(f"wv8c{l}", (104, 360), F8), [(0, 104)],
                360, dtype=F8, name=f"wv8c_{l}", eng=self.nc.sync)[0]
            wobd = self.din(f"wob{l}", (NH, HD, EMB), BF16)
            W["wob"] = []
            for h in range(NH):
                t = wpool.tile([HD, EMB], BF16, name=f"wob_{l}_{h}",
                               tag=f"wob_{l}_{h}")
                self.nc.sync.dma_start(out=t, in_=wobd[h])
                W["wob"].append(t)
            W["wf18"] = self.load_rows(
                wpool, self.din(f"wf18d{l}", (128, 4096), F8), [(0, 128)],
                4096, dtype=F8, name=f"wf18_{l}", eng=self.nc.sync)[0]
            W["wf18c"] = self.load_rows(
                wpool, self.din(f"wf18c{l}", (104, FFD), F8), [(0, 104)],
                FFD, dtype=F8, name=f"wf18c_{l}", eng=self.nc.sync)[0]
            wf28d = self.din(f"wf28d{l}", (8, 128, 2 * EMBP), F8)
            W["wf28"] = []
            for j in range(8):
                t = wpool.tile([128, 2 * EMBP], F8, name=f"wf28_{l}_{j}",
                               tag=f"wf28_{l}_{j}")
                self.nc.sync.dma_start(out=t, in_=wf28d[j])
                W["wf28"].append(t)
            bqkvd = self.din(f"bqkv{l}c", (2 * EMB, 1))
            bqvd = self.din(f"bqkv{l}vr", (1, EMB), F32R)
            bod = self.din(f"bo{l}c", (EMB, 1))
            ln1sd = self.din(f"ln1s{l}c", (EMB, 1))
            ln1bd = self.din(f"ln1b{l}c", (EMB, 1))
            bf1d = self.din(f"bf1_{l}c", (FFD, 1))
            bf2d = self.din(f"bf2_{l}c", (EMB, 1))
            ln2sd = self.din(f"ln2s{l}c", (EMB, 1))
            ln2bd = self.din(f"ln2b{l}c", (EMB, 1))
            if f["use_bqkv"]:
                W["bqkvc"] = self.load_cols(wpool, bqkvd, chunks(2 * EMB, HD),
                                            name=f"bqc{l}", eng=self.nc.sync)
                W["bqv"] = self.load_rows(wpool, bqvd, [(0, 1)], EMB,
                                          dtype=F32R, name=f"bqv{l}", eng=self.nc.sync)[0]
            if f["use_bo"]:
                W["boc"] = self.load_cols(wpool, bod, XCH, name=f"boc{l}", eng=self.nc.sync)
            if not f["ln1_trivial"][l]:
                W["ln1s"] = self.load_cols(wpool, ln1sd, XCH, name=f"ln1s{l}", eng=self.nc.sync)
                W["ln1b"] = self.load_cols(wpool, ln1bd, XCH, name=f"ln1b{l}", eng=self.nc.sync)
            if f["use_bf1"]:
                W["bf1c"] = self.load_cols(wpool, bf1d, chunks(FFD, 128),
                                           name=f"bf1c{l}", eng=self.nc.sync)
            if f["use_bf2"]:
                W["bf2c"] = self.load_cols(wpool, bf2d, XCH, name=f"bf2c{l}", eng=self.nc.sync)
            if not f["ln2_trivial"][l]:
                W["ln2s"] = self.load_cols(wpool, ln2sd, XCH, name=f"ln2s{l}", eng=self.nc.sync)
                W["ln2b"] = self.load_cols(wpool, ln2bd, XCH, name=f"ln2b{l}", eng=self.nc.sync)
            self.W[l] = W

        H = {}
        H["cw1"] = self.load_rows(wpool, self.din("cw1", (EMB, C1)), XCH, C1,
                                  name="cw1", eng=self.nc.sync)
        H["cb1"] = self.load_cols(wpool, self.din("cb1c", (C1, 1)),
                                  chunks(C1, 128), name="cb1", eng=self.nc.sync)
        H["cw2"] = self.load_rows(wpool, self.din("cw2", (C1, C2)),
                                  chunks(C1, 128), C2, name="cw2", eng=self.nc.sync)
        H["cb2"] = self.load_cols(wpool, self.din("cb2c", (C2, 1)),
                                  [(0, C2)], name="cb2", eng=self.nc.sync)
        H["cw3"] = self.load_rows(wpool, self.din("cw3", (C2, NCLS)),
                                  [(0, C2)], NCLS, name="cw3", eng=self.nc.sync)
        H["cb3"] = self.load_cols(wpool, self.din("cb3c", (NCLS, 1)),
                                  [(0, NCLS)], name="cb3", eng=self.nc.sync)
        self.HW = H

    # ---------- phase A/B: embed + nodes ----------
    def phase_embed_nodes(self, xpool, emb):
        nc, tc = self.nc, self.tc
        Tpad = self.Tpad
        f = self.flags
        ones_row = self._ones_row

        kch_df = chunks(DF, 128)
        mch_450 = chunks(H450, 128)
        mch_400 = chunks(NROI, 128)

        xt = [xpool.tile([msz, Tpad], F32R, name=f"xt{mi}", tag=f"xt{mi}")
              for mi, (ms, msz) in enumerate(XCH)]
        x8 = self.x8pool.tile([128, 2 * Tpad], F8, name="x8", tag="x8")
        x8c = self.x8pool.tile([104, Tpad], F8, name="x8c", tag="x8c")
        x8r = x8.rearrange("p (i t) -> p i t", i=2)

        epool = emb["pool"]
        w1t, frt, b1c = emb["w1t"], emb["frt"], emb["b1c"]
        w2t, b2rt, stt = emb["w2t"], emb["b2rt"], emb["stt"]
        with ExitStack() as es:
            epsum = es.enter_context(
                tc.tile_pool(name="embed_ps", bufs=2, space="PSUM"))

            g = []
            for mi, (ms, msz) in enumerate(mch_450):
                ps = epsum.tile([128, NROI], F32, name=f"psA{mi}", tag="psA")
                for ki in range(len(kch_df)):
                    nc.tensor.matmul(ps[:msz], w1t[ki][:, ms : ms + msz],
                                     frt[ki], start=(ki == 0),
                                     stop=(ki == len(kch_df) - 1))
                gt = epool.tile([msz, NROI], F32R, name=f"g{mi}", tag=f"g{mi}")
                nc.scalar.activation(gt, ps[:msz], AF.Gelu, bias=b1c[mi])
                g.append(gt)

            femb = []
            for mi, (ms, msz) in enumerate(mch_400):
                ps = epsum.tile([128, EMB], F32, name=f"psB{mi}", tag="psB")
                nk = len(mch_450)
                for ki in range(nk):
                    nc.tensor.matmul(ps[:msz], g[ki][:, ms : ms + msz],
                                     w2t[ki], start=(ki == 0),
                                     stop=(ki == nk - 1 and not f["use_b2"]))
                if f["use_b2"]:
                    nc.tensor.matmul(ps[:msz], ones_row[:, :msz], b2rt,
                                     start=False, stop=True)
                ft = epool.tile([msz, EMB], F32R, name=f"femb{mi}",
                                tag=f"femb{mi}")
                nc.vector.tensor_copy(ft, ps[:msz])
                femb.append(ft)

            if self.dbg:
                self.debug_dump("femb",
                                [(s, t) for (s, _), t in zip(mch_400, femb)])

            npsum = es.enter_context(
                tc.tile_pool(name="nodes_ps", bufs=3, space="PSUM"))
            for qs, qsz in chunks(Tpad, QCH):
                for mi, (ms, msz) in enumerate(XCH):
                    ps = npsum.tile([128, QCH], F32, name=f"psN{mi}", tag="psN")
                    for ki, (ks, ksz) in enumerate(mch_400):
                        nc.tensor.matmul(ps[:msz, :qsz],
                                         femb[ki][:, ms : ms + msz],
                                         stt[ki][:, qs : qs + qsz],
                                         start=(ki == 0),
                                         stop=(ki == len(mch_400) - 1))
                    nc.vector.tensor_copy(xt[mi][:, qs : qs + qsz],
                                          ps[:msz, :qsz])
                    if mi < 2:
                        nc.gpsimd.tensor_copy(x8r[:, mi, qs : qs + qsz],
                                              xt[mi][:, qs : qs + qsz])
                    else:
                        nc.gpsimd.tensor_copy(x8c[:, qs : qs + qsz],
                                              xt[mi][:, qs : qs + qsz])

            # preload the exp ACT table off the attention critical path
            warm = epool.tile([1, 1], F32)
            nc.scalar.activation(warm, ones_row[:, :1], AF.Exp)

        self._estack.close()
        if self.dbg:
            self.debug_dump("tokens", [(s, t) for (s, _), t in zip(XCH, xt)])
        return xt, x8, x8c

    # ---------- transformer layer ----------
    def phase_layer(self, l, xt, x8, x8c, xpool, pending):
        nc, tc = self.nc, self.tc
        Tpad = self.Tpad
        f = self.flags
        W = self.W[l]

        tch = chunks(Tpad, 128)
        npair = len(tch) // 2
        qhch = chunks(Tpad, QH)
        qch = chunks(Tpad, QCH)
        scale = 1.0 / math.sqrt(HD)

        x8r = x8.rearrange("p (i t) -> p i t", i=2)
        wqk8r = W["wqk8"].rearrange("p (i m) -> p i m", i=2)
        wv8r = W["wv8"].rearrange("p (i m) -> p i m", i=2)

        z = [xpool.tile([msz, Tpad], F32R, name=f"z{l}_{mi}",
                        tag=f"xt{mi}") for mi, (ms, msz) in enumerate(XCH)]
        y = z  # LN1 applies in place

        with ExitStack() as les:
            apool = les.enter_context(tc.tile_pool(name=f"attn{l}", bufs=1))
            exp_pool = les.enter_context(tc.tile_pool(name=f"exp{l}", bufs=3))
            rbpool = les.enter_context(tc.tile_pool(name=f"rb{l}", bufs=1))
            sq_pool = les.enter_context(tc.tile_pool(name=f"sq{l}", bufs=1))
            lnp = les.enter_context(tc.tile_pool(name=f"ln1p{l}", bufs=1))
            y8 = self.x8pool.tile([128, 2 * Tpad], F8, name=f"y8_{l}",
                                  tag="x8")
            y8c = self.x8pool.tile([104, Tpad], F8, name=f"y8c_{l}",
                                   tag="x8c")
            y8r = y8.rearrange("p (i t) -> p i t", i=2)

            # ---- Q/K (90, Tpad) fp8 per head; V pair tiles fp8.  The
            # previous layer's deferred LN2-apply(qh1) is emitted between
            # the x8(qh0)- and x8(qh1)-dependent halves so its stats
            # roundtrip latency hides under the first half. ----
            qkt = {}
            for h in range(NH):
                for nm in ("q", "k"):
                    qkt[nm, h] = apool.tile([HD, Tpad], F8, name=f"{nm}T{h}",
                                            tag=f"{nm}T{h}")
            vxt = {}
            cpool = les.enter_context(tc.tile_pool(name=f"corr{l}", bufs=1))
            # per-layer fp8 weight-residual tiles (tiny corr matmuls only)
            W["dwqk"] = self.load_rows(
                cpool, self.din(f"dwqk{l}", (EMB, 2 * EMB), F8), XCH,
                2 * EMB, dtype=F8, name=f"dwqk{l}", eng=self.nc.sync)
            W["dwv"] = self.load_rows(
                cpool, self.din(f"dwv{l}", (EMB, EMB), F8), XCH, EMB,
                dtype=F8, name=f"dwv{l}", eng=self.nc.sync)
            W["dwf1"] = self.load_rows(
                cpool, self.din(f"dwf1_{l}", (EMB, FFD), F8), XCH, FFD,
                dtype=F8, name=f"dwf1{l}", eng=self.nc.sync)
            W["dwf2"] = self.load_rows(
                cpool, self.din(f"dwf2_{l}", (FFD, EMB), F8),
                chunks(FFD, 128), EMB, dtype=F8, name=f"dwf2{l}", eng=self.nc.sync)

            with ExitStack() as qes:
                qkv_ps = qes.enter_context(
                    tc.tile_pool(name=f"qkvps{l}", bufs=3, space="PSUM"))

                # rank-1 fp8-residual correction: qkv += dW^T @ mean(x),
                # with the token-mean sampled over the first SMPL tokens
                xs = []
                for mi, (ms, msz) in enumerate(XCH):
                    c = cpool.tile([msz, 1], BF16, name=f"xs{mi}",
                                   tag=f"xs{mi}")
                    nc.vector.reduce_sum(c, xt[mi][:, :SMPL], axis=AX.X)
                    xs.append(c)
                qkc = cpool.tile([HD, 8], F32, name="qkc", tag="qkc")
                for oc in range(8):
                    psc = qkv_ps.tile([HD, 1], F32, name="psc", tag="aux",
                                      bufs=2)
                    for ki in range(3):
                        nc.tensor.matmul(
                            psc, W["dwqk"][ki][:, oc * HD : (oc + 1) * HD],
                            xs[ki], start=(ki == 0), stop=(ki == 2))
                    nc.vector.tensor_scalar(qkc[:, oc : oc + 1], psc,
                                            CSC, None, op0=ALU.mult)
                psv = qkv_ps.tile([1, EMB], F32, name="psvr", tag="aux",
                                  bufs=2)
                for ki in range(3):
                    nc.tensor.matmul(psv, xs[ki], W["dwv"][ki],
                                     start=(ki == 0), stop=(ki == 2))
                vrow = cpool.tile([1, EMB], F32R, name="vrow", tag="vrow")
                nc.vector.tensor_scalar(vrow, psv, CSC, None,
                                        op0=ALU.mult)
                if f["use_bqkv"]:
                    for oc in range(8):
                        nc.vector.tensor_scalar(qkc[:, oc : oc + 1],
                                                qkc[:, oc : oc + 1],
                                                W["bqkvc"][oc], None,
                                                op0=ALU.add)
                if self.dbg:
                    self.debug_dump(f"qkc{l}", [(0, qkc)])
                    self.debug_dump(f"vrow{l}", [(0, vrow)])
                    self.debug_dump(f"xs{l}",
                                    [(s, t) for (s, _), t in zip(XCH, xs)])

                def qk_emit(ch_ids):
                    for h in range(NH):
                        for nm, base in (("q", h * HD), ("k", EMB + h * HD)):
                            dst = qkt[nm, h]
                            for qsi in ch_ids:
                                qs, qsz = qch[qsi]
                                ps = qkv_ps.tile([HD, QCH], F32, name="psQK",
                                                 tag="psQK")
                                nc.tensor.matmul(
                                    ps[:, :qsz],
                                    wqk8r[:, :, base : base + HD],
                                    x8r[:, :, qs : qs + qsz], perf_mode=DRM,
                                    start=True, stop=False)
                                nc.tensor.matmul(
                                    ps[:, :qsz],
                                    W["wqk8c"][:, base : base + HD],
                                    x8c[:, qs : qs + qsz], start=False,
                                    stop=True)
                                oc = base // HD
                                nc.vector.tensor_scalar(
                                    dst[:, qs : qs + qsz], ps[:, :qsz],
                                    qkc[:, oc : oc + 1], None,
                                    op0=ALU.add)

                def v_emit(parts):
                    for j, par in parts:
                        if j not in vxt:
                            vxt[j] = apool.tile([128, 2 * NH * HD2], F8,
                                                name=f"vx{j}", tag=f"vx{j}")
                        vt4 = vxt[j].rearrange("p (i h d) -> p i h d", i=2,
                                               h=NH)
                        ts = (2 * j + par) * 128
                        ps = qkv_ps.tile([128, EMB], F32, name="psV",
                                         tag="psV")
                        nc.tensor.matmul(
                            ps, x8r[:, :, ts : ts + 128], wv8r[:, :, :EMB],
                            perf_mode=DRM, start=True, stop=False)
                        nc.tensor.matmul(
                            ps, x8c[:, ts : ts + 128], W["wv8c"],
                            start=False, stop=False)
                        if f["use_bqkv"]:
                            nc.tensor.matmul(ps, self._ones_row[:, :128],
                                             W["bqv"], start=False,
                                             stop=False)
                        nc.tensor.matmul(ps, self._ones_row[:, :128], vrow,
                                         start=False, stop=True)
                        nc.vector.tensor_copy(
                            vt4[:, par, :, :HD],
                            ps.rearrange("p (h d) -> p h d", h=NH))
                        nc.gpsimd.memset(vt4[:, par, :, HD : HD + 1], 1.0)
                        nc.gpsimd.memset(vt4[:, par, :, HD + 1 : HD + 2], 0.0)

                allp = [(j, par) for j in range(npair) for par in range(2)]
                p1 = [(j, par) for j, par in allp
                      if (2 * j + par) * 128 + 128 <= QH]
                p2 = [jp for jp in allp if jp not in p1]
                h1_ids = [i for i, (qs, qsz) in enumerate(qch)
                          if qs + qsz <= QH]
                h2_ids = [i for i in range(len(qch)) if i not in h1_ids]
                qk_emit(h1_ids)
                v_emit(p1)
                if pending is not None:
                    pending(qkv_ps)
                qk_emit(h2_ids)
                v_emit(p2)

            vx = [vxt[j] for j in range(npair)]
            self._osbB = {}
            self._osb8 = {}
            self._den = {}

            def attention_qh(att_ps, qhi, qhs, qhsz):
                den_d = self.dscr.tile([NH, 1, qhsz], BF16,
                                       name=f"den{l}_{qhi}")
                self._den[qhi] = den_d
                osbs = []
                for h in range(NH):
                    pso = att_ps.tile([HD2, QH], F32, name="pso",
                                      tag="pso", bufs=1)
                    for j in range(npair):
                        et = exp_pool.tile([128, 2 * QH], F8, name="et",
                                           tag="et", bufs=2)
                        etr = et.rearrange("p (i t) -> p i t", i=2)
                        for par in range(2):
                            kts = (2 * j + par) * 128
                            pss = att_ps.tile([128, QH], F32, name="pss",
                                              tag="pss", bufs=2)
                            for ss, ssz in chunks(qhsz, PCH):
                                nc.tensor.matmul(
                                    pss[:, ss : ss + ssz],
                                    qkt["k", h][:, kts : kts + 128],
                                    qkt["q", h][:,
                                                qhs + ss : qhs + ss + ssz],
                                    start=True, stop=True)
                            nc.scalar.activation(
                                etr[:, par, :qhsz], pss[:, :qhsz], AF.Exp,
                                bias=self._expb, scale=scale)
                        for ss, ssz in chunks(qhsz, PCH):
                            nc.tensor.matmul(
                                pso[:, ss : ss + ssz],
                                vx[j].rearrange(
                                    "p (i x) -> p i x", i=2)[
                                    :, :, h * HD2 : (h + 1) * HD2],
                                etr[:, :, ss : ss + ssz],
                                perf_mode=DRM,
                                start=(j == 0), stop=(j == npair - 1))
                    ob = apool.tile([HD1, QH], BF16, name=f"osbB{h}",
                                    tag=f"osbB{h}", bufs=1)
                    nc.vector.tensor_copy(ob[:, :qhsz], pso[:HD1, :qhsz])
                    nc.sync.dma_start(out=den_d[h],
                                      in_=ob[HD : HD + 1, :qhsz])
                    osbs.append(ob)
                self._osbB[qhi] = osbs

            def post_den(qhi, qhs, qhsz):
                # denominator -> reciprocal roundtrip; per-head normalize
                # fused into the fp8 quantize of the attention numerator
                osbs = self._osbB[qhi]
                den_d = self._den[qhi]
                nwq = qhsz // 16
                d64 = rbpool.tile([64, nwq], BF16, name="d64", tag="d64")
                nc.sync.dma_start(
                    out=d64,
                    in_=den_d.rearrange("h o (p w) -> (h o p) w", p=16))
                df = rbpool.tile([64, nwq], F32R, name="df", tag="df")
                nc.vector.tensor_scalar(df, d64, self.padc, 1.0 / OSCL,
                                        op0=ALU.subtract, op1=ALU.mult)
                r64 = rbpool.tile([64, nwq], BF16, name="r64", tag="r64")
                nc.vector.reciprocal(r64, df)
                rec_d = self.dscr.tile([NH, 16, nwq], BF16,
                                       name=f"rec{l}_{qhi}")
                nc.sync.dma_start(
                    out=rec_d.rearrange("h p w -> (h p) w"), in_=r64)
                o8 = []
                for h in range(NH):
                    t = apool.tile([HD, QH], BF16, name=f"osbN{h}",
                                   tag=f"osbN{h}", bufs=2)
                    o8.append(t)
                self._osb8[qhi] = o8
                for h in range(NH):
                    rb = rbpool.tile([HD, QH], BF16, name=f"rb{h}",
                                     tag=f"rb{h & 1}")
                    nc.sync.dma_start(
                        out=rb[:, :qhsz],
                        in_=rec_d[h].rearrange(
                            "p w -> (p w)").partition_broadcast(HD))
                    nc.vector.tensor_tensor(
                        o8[h][:, :qhsz],
                        osbs[h][:HD, :qhsz], rb[:, :qhsz], op=ALU.mult)

            def post_proj(qhi, qhs, qhsz, ps_pool):
                # out-proj (head-pair DoubleRow) + descale + residual,
                # then LN1 stats (apply is deferred to post_b)
                o8 = self._osb8[qhi]
                for qs0, qsz in chunks(qhsz, QCH):
                    qs = qhs + qs0
                    for mi, (ms, msz) in enumerate(XCH):
                        ps = ps_pool.tile([128, QCH], F32, name="psPJ",
                                          tag="aux", bufs=2)
                        for h in range(NH):
                            nc.tensor.matmul(
                                ps[:msz, :qsz],
                                W["wob"][h][:, ms : ms + msz],
                                o8[h][:, qs0 : qs0 + qsz],
                                start=(h == 0), stop=(h == NH - 1))
                        nc.vector.scalar_tensor_tensor(
                            z[mi][:, qs : qs + qsz], ps[:msz, :qsz],
                            1.0 / OSCL, xt[mi][:, qs : qs + qsz],
                            op0=ALU.mult, op1=ALU.add)
                        if f["use_bo"]:
                            nc.vector.tensor_scalar(
                                z[mi][:, qs : qs + qsz],
                                z[mi][:, qs : qs + qsz],
                                W["boc"][mi], None, op0=ALU.add)
                return self.emit_ln_stats(f"ln1_{l}_{qhi}", z, qhs, qhsz,
                                          ps_pool, sq_pool, lnp)

            def post_b(qhi, qhs, qhsz, ab2, ps_pool):
                self.emit_ln_apply(z, y, qhs, qhsz, ps_pool, ab2,
                                   W.get("ln1s"), W.get("ln1b"),
                                   f["ln1_trivial"][l])
                for mi in range(2):
                    nc.gpsimd.tensor_copy(y8r[:, mi, qhs : qhs + qhsz],
                                          y[mi][:, qhs : qhs + qhsz])
                nc.gpsimd.tensor_copy(y8c[:, qhs : qhs + qhsz],
                                      y[2][:, qhs : qhs + qhsz])

            fc = cpool.tile([128, 16], F32, name="fc", tag="fc")
            with ExitStack() as aes:
                att_ps = aes.enter_context(
                    tc.tile_pool(name=f"attps{l}", bufs=1, space="PSUM"))
                attention_qh(att_ps, 0, *qhch[0])
                post_den(0, *qhch[0])
                attention_qh(att_ps, 1, *qhch[1])
                ab0 = post_proj(0, *qhch[0], att_ps)
                post_b(0, *qhch[0], ab0, att_ps)
                # FFN1 correction columns dW1^T @ mean(y) -> gelu bias
                ys = []
                for mi, (ms, msz) in enumerate(XCH):
                    c = cpool.tile([msz, 1], BF16, name=f"ys{mi}",
                                   tag=f"xs{mi}")
                    nc.vector.reduce_sum(c, y[mi][:, :SMPL], axis=AX.X)
                    ys.append(c)
                for fi in range(16):
                    psc = att_ps.tile([128, 1], F32, name="psfc", tag="aux",
                                      bufs=2)
                    fs = fi * 128
                    for ki in range(3):
                        nc.tensor.matmul(psc,
                                         W["dwf1"][ki][:, fs : fs + 128],
                                         ys[ki], start=(ki == 0),
                                         stop=(ki == 2))
                    nc.vector.tensor_scalar(fc[:, fi : fi + 1], psc,
                                            CSC, None, op0=ALU.mult)
                    if f["use_bf1"]:
                        nc.vector.tensor_scalar(fc[:, fi : fi + 1],
                                                fc[:, fi : fi + 1],
                                                W["bf1c"][fi], None,
                                                op0=ALU.add)
                if self.dbg:
                    self.debug_dump(f"fc{l}", [(0, fc)])
                    self.debug_dump(f"ysum{l}",
                                    [(s, t) for (s, _), t in zip(XCH, ys)])
                post_den(1, *qhch[1])

            # ---- FFN; qh1's attention epilogue + LN1 interleave under it
            z2 = [xpool.tile([msz, Tpad], F32R, name=f"z2_{l}_{mi}",
                             tag=f"xt{mi}") for mi, (ms, msz) in enumerate(XCH)]
            xnew = z2  # LN2 applies in place
            x8n = self.x8pool.tile([128, 2 * Tpad], F8, name=f"x8n_{l}",
                                   tag="x8")
            x8nc = self.x8pool.tile([104, Tpad], F8, name=f"x8nc_{l}",
                                    tag="x8c")
            x8nr = x8n.rearrange("p (i t) -> p i t", i=2)
            wf18r = W["wf18"].rearrange("p (i m) -> p i m", i=2)

            with ExitStack() as es:
                f1_ps = es.enter_context(
                    tc.tile_pool(name=f"f1ps{l}", bufs=2, space="PSUM"))
                f2_ps = es.enter_context(
                    tc.tile_pool(name=f"f2ps{l}", bufs=2, space="PSUM"))
                ln2_ps = es.enter_context(
                    tc.tile_pool(name=f"ln2ps{l}", bufs=2, space="PSUM"))
                hpool = es.enter_context(tc.tile_pool(name=f"hp{l}", bufs=2))
                sq2_pool = es.enter_context(tc.tile_pool(name=f"sq2{l}",
                                                         bufs=1))
                lnp2 = es.enter_context(tc.tile_pool(name=f"ln2p{l}",
                                                     bufs=1))

                def ffn1_qh(qhs, qhsz):
                    h8 = [hpool.tile([128, 2 * QH], F8, name=f"h8_{jp}",
                                     tag=f"h8_{jp}", bufs=1)
                          for jp in range(8)]
                    for fi in range(16):
                        fs = fi * 128
                        ps = f1_ps.tile([128, QH], F32, name="psF1",
                                        tag="psF1", bufs=2)
                        for ss, ssz in chunks(qhsz, PCH):
                            nc.tensor.matmul(
                                ps[:, ss : ss + ssz],
                                wf18r[:, :, fs : fs + 128],
                                y8r[:, :, qhs + ss : qhs + ss + ssz],
                                perf_mode=DRM, start=True, stop=False)
                            nc.tensor.matmul(
                                ps[:, ss : ss + ssz],
                                W["wf18c"][:, fs : fs + 128],
                                y8c[:, qhs + ss : qhs + ss + ssz],
                                start=False, stop=True)
                        nc.scalar.activation(
                            h8[fi // 2].rearrange(
                                "p (i t) -> p i t", i=2)[:, fi % 2, :qhsz],
                            ps[:, :qhsz], AF.Gelu, bias=fc[:, fi : fi + 1])
                    return h8

                def ffn2_qh(h8, qhs, qhsz):
                    for qs0, qsz in chunks(qhsz, QCH):
                        qs = qhs + qs0
                        for mi, (ms, msz) in enumerate(XCH):
                            ps2 = f2_ps.tile([128, QCH], F32, name="psF2",
                                             tag="psF2", bufs=2)
                            for jp in range(8):
                                nc.tensor.matmul(
                                    ps2[:msz, :qsz],
                                    W["wf28"][jp].rearrange(
                                        "p (i m) -> p i m",
                                        i=2)[:, :, ms : ms + msz],
                                    h8[jp].rearrange(
                                        "p (i t) -> p i t",
                                        i=2)[:, :, qs0 : qs0 + qsz],
                                    perf_mode=DRM, start=(jp == 0),
                                    stop=(jp == 7))
                            nc.vector.scalar_tensor_tensor(
                                z2[mi][:, qs : qs + qsz], ps2[:msz, :qsz],
                                mc[:msz, mi : mi + 1],
                                y[mi][:, qs : qs + qsz], op0=ALU.add,
                                op1=ALU.add)
                            if f["use_bf2"]:
                                nc.vector.tensor_scalar(
                                    z2[mi][:, qs : qs + qsz],
                                    z2[mi][:, qs : qs + qsz],
                                    W["bf2c"][mi], None, op0=ALU.add)

                h8a = ffn1_qh(*qhch[0])
                ab1 = post_proj(1, *qhch[1], ln2_ps)
                # FFN2 correction columns dW2^T @ mean(h) -> residual bias
                hs = cpool.tile([128, 16], BF16, name="hs", tag="hs")
                for fi in range(16):
                    nc.vector.reduce_sum(
                        hs[:, fi : fi + 1],
                        h8a[fi // 2][:, (fi % 2) * QH : (fi % 2) * QH + SMPL],
                        axis=AX.X)
                mc = cpool.tile([128, 3], F32R, name="mc", tag="mc")
                if self.dbg:
                    nc.vector.memset(mc, 0.0)
                for mi, (ms, msz) in enumerate(XCH):
                    psc = ln2_ps.tile([128, 1], F32, name="psmc", tag="aux",
                                      bufs=2)
                    for kc in range(16):
                        nc.tensor.matmul(psc[:msz],
                                         W["dwf2"][kc][:, ms : ms + msz],
                                         hs[:, kc : kc + 1],
                                         start=(kc == 0), stop=(kc == 15))
                    nc.vector.tensor_scalar(mc[:msz, mi : mi + 1], psc[:msz],
                                            CSC, None, op0=ALU.mult)
                if self.dbg:
                    self.debug_dump(f"mcd{l}", [(0, mc)])
                    self.debug_dump(f"hsd{l}", [(0, hs)])
                ffn2_qh(h8a, *qhch[0])
                post_b(1, *qhch[1], ab1, ln2_ps)
                h8b = ffn1_qh(*qhch[1])
                st0 = self.emit_ln_stats(f"ln2_{l}_0", z2, *qhch[0],
                                         ln2_ps, sq2_pool, lnp2)
                ffn2_qh(h8b, *qhch[1])
                self.emit_ln_apply(z2, xnew, *qhch[0], ln2_ps, st0,
                                   W.get("ln2s"), W.get("ln2b"),
                                   f["ln2_trivial"][l])
                self.emit_x8(xnew, x8nr, x8nc, *qhch[0])
                st1 = self.emit_ln_stats(f"ln2_{l}_1", z2, *qhch[1],
                                         ln2_ps, sq2_pool, lnp2)
                if l + 1 < DEPTH:
                    warm = hpool.tile([1, 1], F32, name="warm", tag="warm")
                    nc.scalar.activation(warm, self._ones_row[:, :1], AF.Exp)

                qhs1, qhsz1 = qhch[1]
                ln2s, ln2b = W.get("ln2s"), W.get("ln2b")
                triv2 = f["ln2_trivial"][l]

                def pending_new(ps_pool):
                    self.emit_ln_apply(z2, xnew, qhs1, qhsz1, ps_pool, st1,
                                       ln2s, ln2b, triv2)
                    self.emit_x8(xnew, x8nr, x8nc, qhs1, qhsz1)

                if self.dbg:
                    # dbg runs apply immediately so the dumps see final data
                    pending_new(ln2_ps)
                    pending_new = None

            if self.dbg:
                self.debug_dump(f"y{l}", [(s, t) for (s, _), t in zip(XCH, y)])
                self.debug_dump(f"x{l + 1}",
                                [(s, t) for (s, _), t in zip(XCH, xnew)])
            return xnew, x8n, x8nc, pending_new

    def emit_x8(self, xnew, x8nr, x8nc, qhs, qhsz):
        nc = self.nc
        for mi in range(2):
            nc.gpsimd.tensor_copy(x8nr[:, mi, qhs : qhs + qhsz],
                                  xnew[mi][:, qhs : qhs + qhsz])
        nc.gpsimd.tensor_copy(x8nc[:, qhs : qhs + qhsz],
                              xnew[2][:, qhs : qhs + qhsz])

    # ---------- layernorm over partition (EMB) axis, one q-half ----------
    def emit_ln_stats(self, name, z, qhs, qhsz, ps_pool, sq_pool, lnp,
                      row_bufs=1):
        nc = self.nc
        inv_d = 1.0 / EMB
        ones_col = self._ones_col
        i32 = mybir.dt.int32
        sum_t = lnp.tile([1, QH], F32, name="sum_t", tag="sum_t",
                         bufs=row_bufs)
        sq_t = lnp.tile([1, QH], F32, name="sq_t", tag="sq_t", bufs=row_bufs)
        a2 = self.lnrows.tile([1, QH], F32R, name="a2", tag="a2", bufs=2)
        b2 = self.lnrows.tile([1, QH], F32R, name="b2", tag="b2", bufs=2)

        for qs0, qsz in chunks(qhsz, QCH):
            qs = qhs + qs0
            psm = ps_pool.tile([1, QCH], F32, name="psm", tag="aux", bufs=2)
            pssq = ps_pool.tile([1, QCH], F32, name="pssq", tag="aux", bufs=2)
            for mi, (ms, msz) in enumerate(XCH):
                sq = sq_pool.tile([msz, QCH], BF16, name="sq", tag=f"sq{mi}")
                nc.gpsimd.tensor_tensor(sq[:, :qsz], z[mi][:, qs : qs + qsz],
                                        z[mi][:, qs : qs + qsz], op=ALU.mult)
                nc.tensor.matmul(psm[:, :qsz], ones_col[:msz, :],
                                 z[mi][:, qs : qs + qsz], start=(mi == 0),
                                 stop=(mi == len(XCH) - 1))
                nc.tensor.matmul(pssq[:, :qsz], self._ones_col_b[:msz, :],
                                 sq[:, :qsz], start=(mi == 0),
                                 stop=(mi == len(XCH) - 1))
            nc.vector.tensor_copy(sum_t[:, qs0 : qs0 + qsz], psm[:, :qsz])
            nc.vector.tensor_copy(sq_t[:, qs0 : qs0 + qsz], pssq[:, :qsz])

        # rows -> 32 partitions via DRAM
        nw = qhsz // 32
        st_d = self.dscr.tile([2, 1, qhsz], F32, name=f"{name}_std")
        nc.sync.dma_start(out=st_d[0], in_=sum_t[:, :qhsz])
        nc.sync.dma_start(out=st_d[1], in_=sq_t[:, :qhsz])
        st32 = lnp.tile([32, 2 * nw], F32, name="st32", tag="st32")
        nc.sync.dma_start(
            out=st32.rearrange("p (i w) -> p i w", i=2),
            in_=st_d.rearrange("i o (p w) -> p i (o w)", p=32))

        mean = lnp.tile([32, nw], F32, name="mean", tag="mean")
        nc.vector.tensor_scalar(mean, st32[:, 0:nw], inv_d, None,
                                op0=ALU.mult)
        v0 = lnp.tile([32, nw], F32, name="v0", tag="v0")
        nc.vector.tensor_scalar(v0, st32[:, nw : 2 * nw], inv_d, EPS,
                                op0=ALU.mult, op1=ALU.add)
        m2 = lnp.tile([32, nw], F32, name="m2", tag="m2")
        nc.vector.tensor_tensor(m2, mean, mean, op=ALU.mult)
        var = lnp.tile([32, nw], F32, name="var", tag="var")
        nc.vector.tensor_tensor(var, v0, m2, op=ALU.subtract)

        # rsqrt via bit-trick seed + 2 Newton iterations (DVE only)
        seed = lnp.tile([32, nw], i32, name="seed", tag="seed")
        nc.vector.tensor_scalar(seed, var.bitcast(i32), 1, None,
                                op0=ALU.logical_shift_right)
        magic = lnp.tile([32, nw], i32, name="magic", tag="magic")
        nc.vector.memset(magic, 0x5F3759DF)
        y0 = lnp.tile([32, nw], i32, name="y0", tag="y0")
        nc.vector.tensor_tensor(y0, magic, seed, op=ALU.subtract)
        yv = y0.bitcast(F32)
        t1 = lnp.tile([32, nw], F32, name="t1", tag="t1")
        ab = lnp.tile([32, 2 * nw], F32R, name="ab", tag="ab")
        for it in range(2):
            nc.vector.tensor_tensor(t1, var, yv, op=ALU.mult)
            nc.vector.tensor_tensor(t1, t1, yv, op=ALU.mult)
            nc.vector.tensor_scalar(t1, t1, -0.5, 1.5, op0=ALU.mult,
                                    op1=ALU.add)
            if it == 0:
                nc.vector.tensor_tensor(yv, yv, t1, op=ALU.mult)
            else:
                nc.vector.tensor_tensor(ab[:, 0:nw], yv, t1, op=ALU.mult)
        nc.vector.tensor_tensor(ab[:, nw : 2 * nw], mean, ab[:, 0:nw],
                                op=ALU.mult)
        nc.vector.tensor_scalar(ab[:, nw : 2 * nw], ab[:, nw : 2 * nw],
                                -1.0, None, op0=ALU.mult)

        ab_d = self.dscr.tile([32, 2, nw], F32R, name=f"{name}_abd")
        nc.sync.dma_start(out=ab_d, in_=ab.rearrange("p (i w) -> p i w", i=2))
        for i, t in enumerate((a2, b2)):
            nc.sync.dma_start(
                out=t[:, :qhsz].rearrange("o (p w) -> o p w", p=32),
                in_=ab_d[:, i : i + 1, :].rearrange("p i w -> i p w"))

        return (a2, b2)

    def emit_ln_apply(self, z, y, qhs, qhsz, ps_pool, ab2, sc, bc,
                      trivial):
        nc = self.nc
        ones_row = self._ones_row
        a2, b2 = ab2
        for qs0, qsz in chunks(qhsz, QCH):
            qs = qhs + qs0
            psa = ps_pool.tile([128, QCH], F32, name="psa", tag="aux",
                               bufs=2)
            psb = ps_pool.tile([128, QCH], F32, name="psb", tag="aux",
                               bufs=2)
            nc.tensor.matmul(psa[:, :qsz], ones_row[:, :128],
                             a2[:, qs0 : qs0 + qsz], start=True, stop=True)
            nc.tensor.matmul(psb[:, :qsz], ones_row[:, :128],
                             b2[:, qs0 : qs0 + qsz], start=True, stop=True)
            for mi, (ms, msz) in enumerate(XCH):
                nc.vector.tensor_tensor(y[mi][:, qs : qs + qsz],
                                        z[mi][:, qs : qs + qsz],
                                        psa[:msz, :qsz], op=ALU.mult)
                nc.vector.tensor_tensor(y[mi][:, qs : qs + qsz],
                                        y[mi][:, qs : qs + qsz],
                                        psb[:msz, :qsz], op=ALU.add)
                if not trivial:
                    nc.vector.tensor_scalar(y[mi][:, qs : qs + qsz],
                                            y[mi][:, qs : qs + qsz],
                                            sc[mi], bc[mi], op0=ALU.mult,
                                            op1=ALU.add)
                if self.Tpad > self.T and qs + qsz > self.T:
                    # keep zero-padded tokens exactly zero so the pad-key
                    # denominator correction stays exact in later layers
                    nc.vector.memset(
                        y[mi][:, max(qs, self.T) : qs + qsz].bitcast(F32),
                        0.0)

    # ---------- head ----------
    def phase_head(self, xt, pending=None):
        nc, tc = self.nc, self.tc
        T = self.T
        H = self.HW

        outd = self.dout("out", (NCLS, 1))

        with ExitStack() as es:
            hpool = es.enter_context(tc.tile_pool(name="head", bufs=1))
            hps = es.enter_context(
                tc.tile_pool(name="head_ps", bufs=2, space="PSUM"))

            if pending is not None:
                pending(hps)

            hmean = []
            for mi, (ms, msz) in enumerate(XCH):
                hm = hpool.tile([msz, 1], F32, name=f"hm{mi}", tag=f"hm{mi}")
                nc.vector.reduce_sum(hm, xt[mi][:, :T], axis=AX.X)
                nc.vector.tensor_scalar(hm, hm, 1.0 / T, None, op0=ALU.mult)
                hmean.append(hm)
            if self.dbg:
                self.debug_dump("hmean",
                                [(s, t) for (s, _), t in zip(XCH, hmean)])

            h1 = []
            for mi, (ms, msz) in enumerate(chunks(C1, 128)):
                ps = hps.tile([128, 1], F32, name=f"psH1_{mi}", tag="psH")
                for ki in range(len(XCH)):
                    nc.tensor.matmul(ps[:msz], H["cw1"][ki][:, ms : ms + msz],
                                     hmean[ki], start=(ki == 0),
                                     stop=(ki == len(XCH) - 1))
                ht = hpool.tile([msz, 1], F32, name=f"h1_{mi}", tag=f"h1_{mi}")
                nc.scalar.activation(ht, ps[:msz], AF.Gelu, bias=H["cb1"][mi])
                h1.append(ht)

            ps = hps.tile([128, 1], F32, name="psH2", tag="psH")
            for ki in range(len(H["cw2"])):
                nc.tensor.matmul(ps[:C2], H["cw2"][ki], h1[ki],
                                 start=(ki == 0),
                                 stop=(ki == len(H["cw2"]) - 1))
            h2 = hpool.tile([C2, 1], F32)
            nc.scalar.activation(h2, ps[:C2], AF.Relu, bias=H["cb2"][0])

            ps3 = hps.tile([128, 1], F32, name="psH3", tag="psH")
            nc.tensor.matmul(ps3[:NCLS], H["cw3"][0], h2, start=True,
                             stop=True)
            res = hpool.tile([NCLS, 1], F32)
            nc.scalar.activation(res, ps3[:NCLS], AF.Identity,
                                 bias=H["cb3"][0])
            nc.sync.dma_start(out=outd, in_=res)


# ---------------------------------------------------------------------------
# Host side
# ---------------------------------------------------------------------------

def _build_counts(C):
    """S[b, r, n] = #{v in win(n): C[b, v] == r} for r in 0..NROI."""
    Bn = C.shape[0]
    S = np.zeros((Bn, NROI + 1, NB), np.int32)
    b_idx = np.arange(Bn)[:, None]
    n_idx = np.arange(NB)[None, :]
    for di in range(KS):
        for dj in range(KS):
            for dk in range(KS):
                sub = C[:, di : di + 2 * (NBLK - 1) + 1 : ST,
                        dj : dj + 2 * (NBLK - 1) + 1 : ST,
                        dk : dk + 2 * (NBLK - 1) + 1 : ST].reshape(Bn, NB)
                np.add.at(S, (b_idx, sub, n_idx), 1)
    return S


def _f8(x):
    return np.ascontiguousarray(np.asarray(x, np.float32).astype(NPF8))


def _bf(x):
    return np.ascontiguousarray(
        np.asarray(x, np.float32).astype(ml_dtypes.bfloat16))


def _pack_dr(w, k0, ksz=256):
    """Pack rows [k0, k0+256) of w into DoubleRow layout [128, 2*M] fp8."""
    M = w.shape[1]
    out = np.empty((128, 2, M), np.float32)
    out[:, 0, :] = w[k0 : k0 + 128]
    out[:, 1, :] = w[k0 + 128 : k0 + 256]
    return _f8(out.reshape(128, 2 * M))


def host_prepare(inputs):
    inp = {k: np.asarray(v) for k, v in inputs.items()}
    F_roi = inp["F_roi"].astype(np.float32)
    C = inp["C"].astype(np.int64)

    S = _build_counts(C)
    valid = S[:, 1:, :].sum(axis=(0, 1)) > 0
    vidx = np.nonzero(valid)[0]
    T = int(len(vidx))
    Tpad = ((T + 255) // 256) * 256
    s_t = np.zeros((C.shape[0], NROI, Tpad), np.float32)
    s_t[:, :, :T] = S[:, 1:, :][:, :, vidx].astype(np.float32)

    f32 = lambda x: np.ascontiguousarray(np.asarray(x), dtype=np.float32)
    col = lambda x: f32(x).reshape(-1, 1)
    row = lambda x: f32(x).reshape(1, -1)

    shared = {
        "w1": f32(inp["ffn_w1"]), "b1c": col(inp["ffn_b1"]),
        "w2": f32(inp["ffn_w2"]), "b2r": row(inp["ffn_b2"]),
        "cw1": f32(inp["cw1"]), "cb1c": col(inp["cb1"]),
        "cw2": f32(inp["cw2"]), "cb2c": col(inp["cb2"]),
        "cw3": f32(inp["cw3"]), "cb3c": col(inp["cb3"]),
        "ones_col": np.ones((128, 1), np.float32),
        "ones_row": np.ones((1, 128), np.float32),
    }
    for l in range(DEPTH):
        wqkv = f32(inp["wqkv"][l])            # (360, 1080)
        pk = np.empty((128, 2, 3 * EMB), np.float32)
        pk[:, 0] = wqkv[0:128]
        pk[:, 1] = wqkv[128:256]
        shared[f"wqk8d{l}"] = _f8(pk[:, :, : 2 * EMB].reshape(128, -1))
        pkv = np.zeros((128, 2, EMBP), np.float32)
        pkv[:, :, :EMB] = pk[:, :, 2 * EMB :]
        shared[f"wv8d{l}"] = _f8(pkv.reshape(128, -1))
        shared[f"wqk8c{l}"] = _f8(wqkv[256:, : 2 * EMB])
        shared[f"wv8c{l}"] = _f8(wqkv[256:, 2 * EMB :])
        wo = f32(inp["wo"][l])                # (360, 360)
        shared[f"wob{l}"] = _bf(wo.reshape(NH, HD, EMB))
        wf1 = f32(inp["wf1"][l])              # (360, 2048)
        shared[f"wf18d{l}"] = _pack_dr(wf1, 0)
        shared[f"wf18c{l}"] = _f8(wf1[256:])
        wf2 = f32(inp["wf2"][l])              # (2048, 360)
        w28 = np.zeros((8, 128, 2, EMBP), np.float32)
        for j in range(8):
            w28[j, :, 0, :EMB] = wf2[256 * j : 256 * j + 128]
            w28[j, :, 1, :EMB] = wf2[256 * j + 128 : 256 * j + 256]
        shared[f"wf28d{l}"] = _f8(w28.reshape(8, 128, 2 * EMBP))
        # fp8-weight residuals for the rank-1 token-mean correction,
        # scaled UP by DSC so the tiny residuals stay out of fp8's
        # flush-to-zero range; the device folds 1/(DSC*SMPL) back in
        d8 = lambda w: _f8((w - _f8(w).astype(np.float32)) * DSC)
        shared[f"dwqk{l}"] = d8(wqkv[:, : 2 * EMB])
        shared[f"dwv{l}"] = d8(wqkv[:, 2 * EMB :])
        shared[f"dwf1_{l}"] = d8(wf1)
        shared[f"dwf2_{l}"] = d8(wf2)

        shared[f"bqkv{l}c"] = col(inp["bqkv"][l][: 2 * EMB])
        shared[f"bqkv{l}vr"] = row(inp["bqkv"][l][2 * EMB :])
        shared[f"bo{l}c"] = col(inp["bo"][l])
        shared[f"ln1s{l}c"] = col(inp["ln1_s"][l])
        shared[f"ln1b{l}c"] = col(inp["ln1_b"][l])
        shared[f"bf1_{l}c"] = col(inp["bf1"][l])
        shared[f"bf2_{l}c"] = col(inp["bf2"][l])
        shared[f"ln2s{l}c"] = col(inp["ln2_s"][l])
        shared[f"ln2b{l}c"] = col(inp["ln2_b"][l])

    flags = {
        "use_b2": bool(np.any(np.asarray(inp["ffn_b2"]) != 0)),
        "use_bqkv": bool(np.any(np.asarray(inp["bqkv"]) != 0)),
        "use_bo": bool(np.any(np.asarray(inp["bo"]) != 0)),
        "use_bf1": bool(np.any(np.asarray(inp["bf1"]) != 0)),
        "use_bf2": bool(np.any(np.asarray(inp["bf2"]) != 0)),
        "ln1_trivial": [bool(np.all(np.asarray(inp["ln1_s"][l]) == 1)
                             and np.all(np.asarray(inp["ln1_b"][l]) == 0))
                        for l in range(DEPTH)],
        "ln2_trivial": [bool(np.all(np.asarray(inp["ln2_s"][l]) == 1)
                             and np.all(np.asarray(inp["ln2_b"][l]) == 0))
                        for l in range(DEPTH)],
    }

    in_maps = []
    for b in range(F_roi.shape[0]):
        m = dict(shared)
        m["f_roiT"] = np.ascontiguousarray(F_roi[b].T)
        m["s_t"] = np.ascontiguousarray(s_t[b])
        in_maps.append(m)
    return in_maps, T, Tpad, flags


def build_program(T, Tpad, flags, dbg=False):
    nc = bacc.Bacc("TRN2", target_bir_lowering=False, debug=False,
                   enable_asserts=False, num_devices=B)
    with tile.TileContext(nc) as tc:
        with nc.allow_low_precision("fp8/bf16 matmul operand plumbing"):
            with ExitStack() as ctx:
                bld = Builder(nc, tc, ctx, T, Tpad, flags, dbg=dbg)
                bld.build()
    nc.compile()
    return nc


def kernel(**inputs):
    in_maps, T, Tpad, flags = host_prepare(inputs)
    nc = build_program(T, Tpad, flags)
    res = run_bass_kernel_spmd(nc, in_maps, core_ids=list(range(len(in_maps))))
    out = np.stack([r["out"].reshape(NCLS) for r in res.results])
    return out.astype(np.float32)


# revision 43
# speedup vs baseline: 1.0155x; 1.0155x over previous
"""AtlasFreeBrainTransformer Trainium2 kernel (fp8 DoubleRow version).

Host contract: kernel(**inputs) takes the FULL unsharded inputs of
reference.setup_inputs() and returns the FULL (B, 2) float32 output.

Sharding: data-parallel over batch B=8 across the 8 NeuronCores (one
batch element per core, weights replicated, no collectives).  The
valid-node mask reduces over batch; it is computed on the host directly
from the integer index tensor C, so no cross-core reduction is needed.

The gather + reduce_window is collapsed into one matmul with a host-built
integer count matrix S (see _build_counts).

Device pipeline per core (all-transposed layout, x_T is (EMB, T)):
  embed FFN (fp32r, exact) -> nodes via S contraction (fp32r, exact)
  -> DEPTH transformer layers where every big matmul runs in fp8e4m3,
     most in DoubleRow perf mode (2x128 contraction per pass at 0.5
     cyc/row):
       qkv (DR), scores (fp8), exp with constant shift so the softmax
       numerator/denominator fit fp8, AV with key-tile pairs (DR),
       per-head normalize fused into the fp8 quantize copy, out-proj
       with head pairs (DR), FFN1/FFN2 (DR).
     Residual stream stays fp32r.  LayerNorm stats via ones-matmuls +
     DRAM-roundtrip reshape + rsqrt bit trick (as before).
  -> mean over tokens -> 3-layer classifier head (fp32) -> logits.

Elementwise work is split DVE/Pool(gpsimd) to keep the ACT engine free
for exp/gelu (the bottleneck).
"""

import sys

sys.path.insert(0, "/opt/trn_rl_repo")

import math
from contextlib import ExitStack

import numpy as np
import ml_dtypes

import concourse.bass as bass
import concourse.tile as tile
from concourse import bacc, mybir
from concourse.bass_utils import run_bass_kernel_spmd

F32 = mybir.dt.float32
F32R = mybir.dt.float32r
BF16 = mybir.dt.bfloat16
F8 = mybir.dt.float8e4
AF = mybir.ActivationFunctionType
ALU = mybir.AluOpType
AX = mybir.AxisListType
DRM = mybir.MatmulPerfMode.DoubleRow
NPF8 = ml_dtypes.float8_e4m3

# Model dims (hardcoded per problem spec)
B, NROI, DF, G, EMB, NH, HD, FFD, DEPTH = 8, 400, 512, 25, 360, 4, 90, 2048, 2
KS, ST = 3, 2
NBLK = (G - KS) // ST + 1          # 12
NB = NBLK ** 3                     # 1728 nodes
EPS = 1e-5
H450 = 450
C1, C2, NCLS = 256, 128, 2
HD1 = HD + 1                       # 91 (ones row for the denominator)
HD2 = HD + 2                       # 92: even block so DoubleRow lowers
EMBP = 368   # EMB padded so DoubleRow weight strides stay 16B-aligned

QCH = 448    # fp32 psum chunk that fits one 2KB bank
QH = 896     # attention / LN q chunk (2 banks)
PCH = 512    # matmul sub-chunk inside a 2-bank psum tile (bank boundary)
EXPS = -1.5  # exp(score*scale + EXPS): keeps softmax terms in fp8 range
OSCL = 64.0  # osbN = O * OSCL / den so fp8/bf16 sees ~unit-scale values
SMPL = 448   # token-sample count for the rank-1 mean corrections
DSC = 256.0  # fp8 residual-weight upscale (undone on device)
CSC = 1.0 / (DSC * SMPL)

# EMB row chunks: A,B are the DoubleRow contraction pair, C is the tail
XCH = [(0, 128), (128, 128), (256, 104)]


def chunks(total, size):
    out = []
    s = 0
    while s < total:
        out.append((s, min(size, total - s)))
        s += size
    return out


class Builder:
    def __init__(self, nc, tc, ctx, T, Tpad, flags, dbg=False):
        self.nc = nc
        self.tc = tc
        self.ctx = ctx
        self.T = T
        self.Tpad = Tpad
        self.flags = flags
        self.dbg = dbg
        self.dram = {}
        # fp8-rounded pad-key softmax term, dequantized: the device AV
        # matmul accumulates exactly this value per zero-padded key
        self.padc = float(
            np.float32(np.exp(np.float32(EXPS))).astype(NPF8).astype(np.float32)
        ) * (Tpad - T)

    # ---------- dram declarations ----------
    def din(self, name, shape, dtype=F32):
        t = self.nc.dram_tensor(name, list(shape), dtype, kind="ExternalInput")
        self.dram[name] = t.ap()
        return self.dram[name]

    def dout(self, name, shape, dtype=F32):
        t = self.nc.dram_tensor(name, list(shape), dtype, kind="ExternalOutput")
        self.dram[name] = t.ap()
        return self.dram[name]

    def debug_dump(self, name, parts):
        if not self.dbg:
            return
        rows = max(s + ap.shape[0] for s, ap in parts)
        cols = parts[0][1].shape[1] if len(parts[0][1].shape) > 1 else 1
        d = self.dout(f"dbg_{name}", (rows, cols), parts[0][1].dtype)
        for s, ap in parts:
            self.nc.sync.dma_start(out=d[s : s + ap.shape[0], :], in_=ap)

    # ---------- small helpers ----------
    def load_rows(self, pool, dram_ap, row_chunks, cols, dtype=F32, name="w",
                  eng=None):
        tiles = []
        eng = eng or self.nc.scalar
        for i, (s, sz) in enumerate(row_chunks):
            t = pool.tile([sz, cols], dtype, name=f"{name}{i}", tag=f"{name}{i}")
            eng.dma_start(out=t, in_=dram_ap[s : s + sz, :])
            tiles.append(t)
        return tiles

    def load_cols(self, pool, dram_ap, row_chunks, dtype=F32, name="c",
                  eng=None):
        return self.load_rows(pool, dram_ap, row_chunks, 1, dtype=dtype,
                              name=name, eng=eng)

    # ---------- build ----------
    def build(self):
        nc, tc, ctx = self.nc, self.tc, self.ctx

        consts = ctx.enter_context(tc.tile_pool(name="consts", bufs=1))
        self._ones_col = self.load_rows(
            consts, self.din("ones_col", (128, 1), F32R), [(0, 128)], 1,
            dtype=F32R, name="ones_col")[0]
        self._ones_row = self.load_rows(
            consts, self.din("ones_row", (1, 128), F32R), [(0, 1)], 128,
            dtype=F32R, name="ones_row")[0]
        self._expb = consts.tile([128, 1], F32, name="expb", tag="expb")
        nc.vector.memset(self._expb, EXPS)
        self._ones_col_b = consts.tile([128, 1], BF16, name="ones_col_b",
                                       tag="ones_col_b")
        nc.vector.memset(self._ones_col_b, 1.0)
        self.dscr = ctx.enter_context(
            tc.tile_pool(name="dscr", bufs=1, space="DRAM"))
        # persistent rows for the pipelined LN apply (a2/b2 survive into the
        # next phase where the deferred apply is emitted)
        self.lnrows = ctx.enter_context(tc.tile_pool(name="lnrows", bufs=2))

        # persistent pools first (pool lifetimes must nest LIFO); the
        # embed pool opens last so it can close right after the embed
        # phase.  Embed weight DMAs are issued before the big transformer
        # weight DMAs so the first phase is never stuck behind them.
        wpool = ctx.enter_context(tc.tile_pool(name="wpool", bufs=1))
        xpool = ctx.enter_context(tc.tile_pool(name="xpool", bufs=2))
        x8pool = ctx.enter_context(tc.tile_pool(name="x8pool", bufs=2))
        self.x8pool = x8pool
        self._estack = ExitStack()
        epool = self._estack.enter_context(tc.tile_pool(name="embed", bufs=1))
        emb = self.load_embed(epool)
        self.load_weights(wpool)

        xt, x8, x8c = self.phase_embed_nodes(xpool, emb)

        pending = None
        for l in range(DEPTH):
            xt, x8, x8c, pending = self.phase_layer(l, xt, x8, x8c, xpool,
                                                    pending)

        self.phase_head(xt, pending)

    def load_embed(self, epool):
        Tpad = self.Tpad
        w1d = self.din("w1", (DF, H450), F32R)
        b1d = self.din("b1c", (H450, 1))
        w2d = self.din("w2", (H450, EMB), F32R)
        b2r = self.din("b2r", (1, EMB), F32R)
        frd = self.din("f_roiT", (DF, NROI), F32R)
        std = self.din("s_t", (NROI, Tpad), F32R)
        kch_df = chunks(DF, 128)
        mch_450 = chunks(H450, 128)
        mch_400 = chunks(NROI, 128)
        emb = {}
        emb["w1t"] = self.load_rows(epool, w1d, kch_df, H450, dtype=F32R,
                                    name="w1t")
        emb["frt"] = self.load_rows(epool, frd, kch_df, NROI, dtype=F32R,
                                    name="frt", eng=self.nc.sync)
        emb["b1c"] = self.load_cols(epool, b1d, mch_450, name="b1c",
                                    eng=self.nc.sync)
        emb["w2t"] = self.load_rows(epool, w2d, mch_450, EMB, dtype=F32R,
                                    name="w2t")
        emb["b2rt"] = self.load_rows(epool, b2r, [(0, 1)], EMB, dtype=F32R,
                                     name="b2rt")[0]
        emb["stt"] = self.load_rows(epool, std, mch_400, Tpad, dtype=F32R,
                                    name="stt")
        emb["pool"] = epool
        return emb

    def load_weights(self, wpool):
        f = self.flags
        self.W = {}
        for l in range(DEPTH):
            W = {}
            W["wqk8"] = self.load_rows(
                wpool, self.din(f"wqk8d{l}", (128, 1440), F8), [(0, 128)],
                1440, dtype=F8, name=f"wqk8_{l}", eng=self.nc.sync)[0]
            W["wqk8c"] = self.load_rows(
                wpool, self.din(f"wqk8c{l}", (104, 720), F8), [(0, 104)],
                720, dtype=F8, name=f"wqk8c_{l}", eng=self.nc.sync)[0]
            W["wv8"] = self.load_rows(
                wpool, self.din(f"wv8d{l}", (128, 2 * EMBP), F8), [(0, 128)],
                2 * EMBP, dtype=F8, name=f"wv8_{l}", eng=self.nc.sync)[0]
            W["wv8c"] = self.load_rows(
                wpool, self.din(f"wv8c{l}", (104, 360), F8), [(0, 104)],
                360, dtype=F8, name=f"wv8c_{l}", eng=self.nc.sync)[0]
            wobd = self.din(f"wob{l}", (NH, HD, EMB), BF16)
            W["wob"] = []
            for h in range(NH):
                t = wpool.tile([HD, EMB], BF16, name=f"wob_{l}_{h}",
                               tag=f"wob_{l}_{h}")
                self.nc.sync.dma_start(out=t, in_=wobd[h])
                W["wob"].append(t)
            W["wf18"] = self.load_rows(
                wpool, self.din(f"wf18d{l}", (128, 4096), F8), [(0, 128)],
                4096, dtype=F8, name=f"wf18_{l}", eng=self.nc.sync)[0]
            W["wf18c"] = self.load_rows(
                wpool, self.din(f"wf18c{l}", (104, FFD), F8), [(0, 104)],
                FFD, dtype=F8, name=f"wf18c_{l}", eng=self.nc.sync)[0]
            wf28d = self.din(f"wf28d{l}", (8, 128, 2 * EMBP), F8)
            W["wf28"] = []
            for j in range(8):
                t = wpool.tile([128, 2 * EMBP], F8, name=f"wf28_{l}_{j}",
                               tag=f"wf28_{l}_{j}")
                self.nc.sync.dma_start(out=t, in_=wf28d[j])
                W["wf28"].append(t)
            bqkvd = self.din(f"bqkv{l}c", (2 * EMB, 1))
            bqvd = self.din(f"bqkv{l}vr", (1, EMB), F32R)
            bod = self.din(f"bo{l}c", (EMB, 1))
            ln1sd = self.din(f"ln1s{l}c", (EMB, 1))
            ln1bd = self.din(f"ln1b{l}c", (EMB, 1))
            bf1d = self.din(f"bf1_{l}c", (FFD, 1))
            bf2d = self.din(f"bf2_{l}c", (EMB, 1))
            ln2sd = self.din(f"ln2s{l}c", (EMB, 1))
            ln2bd = self.din(f"ln2b{l}c", (EMB, 1))
            if f["use_bqkv"]:
                W["bqkvc"] = self.load_cols(wpool, bqkvd, chunks(2 * EMB, HD),
                                            name=f"bqc{l}", eng=self.nc.sync)
                W["bqv"] = self.load_rows(wpool, bqvd, [(0, 1)], EMB,
                                          dtype=F32R, name=f"bqv{l}", eng=self.nc.sync)[0]
            if f["use_bo"]:
                W["boc"] = self.load_cols(wpool, bod, XCH, name=f"boc{l}", eng=self.nc.sync)
            if not f["ln1_trivial"][l]:
                W["ln1s"] = self.load_cols(wpool, ln1sd, XCH, name=f"ln1s{l}", eng=self.nc.sync)
                W["ln1b"] = self.load_cols(wpool, ln1bd, XCH, name=f"ln1b{l}", eng=self.nc.sync)
            if f["use_bf1"]:
                W["bf1c"] = self.load_cols(wpool, bf1d, chunks(FFD, 128),
                                           name=f"bf1c{l}", eng=self.nc.sync)
            if f["use_bf2"]:
                W["bf2c"] = self.load_cols(wpool, bf2d, XCH, name=f"bf2c{l}", eng=self.nc.sync)
            if not f["ln2_trivial"][l]:
                W["ln2s"] = self.load_cols(wpool, ln2sd, XCH, name=f"ln2s{l}", eng=self.nc.sync)
                W["ln2b"] = self.load_cols(wpool, ln2bd, XCH, name=f"ln2b{l}", eng=self.nc.sync)
            self.W[l] = W

        H = {}
        H["cw1"] = self.load_rows(wpool, self.din("cw1", (EMB, C1)), XCH, C1,
                                  name="cw1", eng=self.nc.sync)
        H["cb1"] = self.load_cols(wpool, self.din("cb1c", (C1, 1)),
                                  chunks(C1, 128), name="cb1", eng=self.nc.sync)
        H["cw2"] = self.load_rows(wpool, self.din("cw2", (C1, C2)),
                                  chunks(C1, 128), C2, name="cw2", eng=self.nc.sync)
        H["cb2"] = self.load_cols(wpool, self.din("cb2c", (C2, 1)),
                                  [(0, C2)], name="cb2", eng=self.nc.sync)
        H["cw3"] = self.load_rows(wpool, self.din("cw3", (C2, NCLS)),
                                  [(0, C2)], NCLS, name="cw3", eng=self.nc.sync)
        H["cb3"] = self.load_cols(wpool, self.din("cb3c", (NCLS, 1)),
                                  [(0, NCLS)], name="cb3", eng=self.nc.sync)
        self.HW = H

    # ---------- phase A/B: embed + nodes ----------
    def phase_embed_nodes(self, xpool, emb):
        nc, tc = self.nc, self.tc
        Tpad = self.Tpad
        f = self.flags
        ones_row = self._ones_row

        kch_df = chunks(DF, 128)
        mch_450 = chunks(H450, 128)
        mch_400 = chunks(NROI, 128)

        xt = [xpool.tile([msz, Tpad], F32R, name=f"xt{mi}", tag=f"xt{mi}")
              for mi, (ms, msz) in enumerate(XCH)]
        x8 = self.x8pool.tile([128, 2 * Tpad], F8, name="x8", tag="x8")
        x8c = self.x8pool.tile([104, Tpad], F8, name="x8c", tag="x8c")
        x8r = x8.rearrange("p (i t) -> p i t", i=2)

        epool = emb["pool"]
        w1t, frt, b1c = emb["w1t"], emb["frt"], emb["b1c"]
        w2t, b2rt, stt = emb["w2t"], emb["b2rt"], emb["stt"]
        with ExitStack() as es:
            epsum = es.enter_context(
                tc.tile_pool(name="embed_ps", bufs=2, space="PSUM"))

            g = []
            for mi, (ms, msz) in enumerate(mch_450):
                ps = epsum.tile([128, NROI], F32, name=f"psA{mi}", tag="psA")
                for ki in range(len(kch_df)):
                    nc.tensor.matmul(ps[:msz], w1t[ki][:, ms : ms + msz],
                                     frt[ki], start=(ki == 0),
                                     stop=(ki == len(kch_df) - 1))
                gt = epool.tile([msz, NROI], F32R, name=f"g{mi}", tag=f"g{mi}")
                nc.scalar.activation(gt, ps[:msz], AF.Gelu, bias=b1c[mi])
                g.append(gt)

            femb = []
            for mi, (ms, msz) in enumerate(mch_400):
                ps = epsum.tile([128, EMB], F32, name=f"psB{mi}", tag="psB")
                nk = len(mch_450)
                for ki in range(nk):
                    nc.tensor.matmul(ps[:msz], g[ki][:, ms : ms + msz],
                                     w2t[ki], start=(ki == 0),
                                     stop=(ki == nk - 1 and not f["use_b2"]))
                if f["use_b2"]:
                    nc.tensor.matmul(ps[:msz], ones_row[:, :msz], b2rt,
                                     start=False, stop=True)
                ft = epool.tile([msz, EMB], F32R, name=f"femb{mi}",
                                tag=f"femb{mi}")
                nc.vector.tensor_copy(ft, ps[:msz])
                femb.append(ft)

            if self.dbg:
                self.debug_dump("femb",
                                [(s, t) for (s, _), t in zip(mch_400, femb)])

            npsum = es.enter_context(
                tc.tile_pool(name="nodes_ps", bufs=3, space="PSUM"))
            for qs, qsz in chunks(Tpad, QCH):
                for mi, (ms, msz) in enumerate(XCH):
                    ps = npsum.tile([128, QCH], F32, name=f"psN{mi}", tag="psN")
                    for ki, (ks, ksz) in enumerate(mch_400):
                        nc.tensor.matmul(ps[:msz, :qsz],
                                         femb[ki][:, ms : ms + msz],
                                         stt[ki][:, qs : qs + qsz],
                                         start=(ki == 0),
                                         stop=(ki == len(mch_400) - 1))
                    nc.vector.tensor_copy(xt[mi][:, qs : qs + qsz],
                                          ps[:msz, :qsz])
                    if mi < 2:
                        nc.gpsimd.tensor_copy(x8r[:, mi, qs : qs + qsz],
                                              xt[mi][:, qs : qs + qsz])
                    else:
                        nc.gpsimd.tensor_copy(x8c[:, qs : qs + qsz],
                                              xt[mi][:, qs : qs + qsz])

            # preload the exp ACT table off the attention critical path
            warm = epool.tile([1, 1], F32)
            nc.scalar.activation(warm, ones_row[:, :1], AF.Exp)

        self._estack.close()
        if self.dbg:
            self.debug_dump("tokens", [(s, t) for (s, _), t in zip(XCH, xt)])
        return xt, x8, x8c

    # ---------- transformer layer ----------
    def phase_layer(self, l, xt, x8, x8c, xpool, pending):
        nc, tc = self.nc, self.tc
        Tpad = self.Tpad
        f = self.flags
        W = self.W[l]

        tch = chunks(Tpad, 128)
        npair = len(tch) // 2
        qhch = chunks(Tpad, QH)
        qch = chunks(Tpad, QCH)
        scale = 1.0 / math.sqrt(HD)

        x8r = x8.rearrange("p (i t) -> p i t", i=2)
        wqk8r = W["wqk8"].rearrange("p (i m) -> p i m", i=2)
        wv8r = W["wv8"].rearrange("p (i m) -> p i m", i=2)

        z = [xpool.tile([msz, Tpad], F32R, name=f"z{l}_{mi}",
                        tag=f"xt{mi}") for mi, (ms, msz) in enumerate(XCH)]
        y = z  # LN1 applies in place

        with ExitStack() as les:
            apool = les.enter_context(tc.tile_pool(name=f"attn{l}", bufs=1))
            exp_pool = les.enter_context(tc.tile_pool(name=f"exp{l}", bufs=3))
            rbpool = les.enter_context(tc.tile_pool(name=f"rb{l}", bufs=1))
            sq_pool = les.enter_context(tc.tile_pool(name=f"sq{l}", bufs=1))
            lnp = les.enter_context(tc.tile_pool(name=f"ln1p{l}", bufs=1))
            y8 = self.x8pool.tile([128, 2 * Tpad], F8, name=f"y8_{l}",
                                  tag="x8")
            y8c = self.x8pool.tile([104, Tpad], F8, name=f"y8c_{l}",
                                   tag="x8c")
            y8r = y8.rearrange("p (i t) -> p i t", i=2)

            # ---- Q/K (90, Tpad) fp8 per head; V pair tiles fp8.  The
            # previous layer's deferred LN2-apply(qh1) is emitted between
            # the x8(qh0)- and x8(qh1)-dependent halves so its stats
            # roundtrip latency hides under the first half. ----
            qkt = {}
            for h in range(NH):
                for nm in ("q", "k"):
                    qkt[nm, h] = apool.tile([HD, Tpad], F8, name=f"{nm}T{h}",
                                            tag=f"{nm}T{h}")
            vxt = {}
            cpool = les.enter_context(tc.tile_pool(name=f"corr{l}", bufs=1))
            # per-layer fp8 weight-residual tiles (tiny corr matmuls only)
            W["dwqk"] = self.load_rows(
                cpool, self.din(f"dwqk{l}", (EMB, 2 * EMB), F8), XCH,
                2 * EMB, dtype=F8, name=f"dwqk{l}", eng=self.nc.sync)
            W["dwv"] = self.load_rows(
                cpool, self.din(f"dwv{l}", (EMB, EMB), F8), XCH, EMB,
                dtype=F8, name=f"dwv{l}", eng=self.nc.sync)
            W["dwf1"] = self.load_rows(
                cpool, self.din(f"dwf1_{l}", (EMB, FFD), F8), XCH, FFD,
                dtype=F8, name=f"dwf1{l}", eng=self.nc.sync)
            W["dwf2"] = self.load_rows(
                cpool, self.din(f"dwf2_{l}", (FFD, EMB), F8),
                chunks(FFD, 128), EMB, dtype=F8, name=f"dwf2{l}", eng=self.nc.sync)

            with ExitStack() as qes:
                qkv_ps = qes.enter_context(
                    tc.tile_pool(name=f"qkvps{l}", bufs=3, space="PSUM"))

                # rank-1 fp8-residual correction: qkv += dW^T @ mean(x),
                # with the token-mean sampled over the first SMPL tokens
                xs = []
                for mi, (ms, msz) in enumerate(XCH):
                    c = cpool.tile([msz, 1], BF16, name=f"xs{mi}",
                                   tag=f"xs{mi}")
                    nc.vector.reduce_sum(c, xt[mi][:, :SMPL], axis=AX.X)
                    xs.append(c)
                qkc = cpool.tile([HD, 8], F32, name="qkc", tag="qkc")
                for oc in range(8):
                    psc = qkv_ps.tile([HD, 1], F32, name="psc", tag="aux",
                                      bufs=2)
                    for ki in range(3):
                        nc.tensor.matmul(
                            psc, W["dwqk"][ki][:, oc * HD : (oc + 1) * HD],
                            xs[ki], start=(ki == 0), stop=(ki == 2))
                    nc.vector.tensor_scalar(qkc[:, oc : oc + 1], psc,
                                            CSC, None, op0=ALU.mult)
                psv = qkv_ps.tile([1, EMB], F32, name="psvr", tag="aux",
                                  bufs=2)
                for ki in range(3):
                    nc.tensor.matmul(psv, xs[ki], W["dwv"][ki],
                                     start=(ki == 0), stop=(ki == 2))
                vrow = cpool.tile([1, EMB], F32R, name="vrow", tag="vrow")
                nc.vector.tensor_scalar(vrow, psv, CSC, None,
                                        op0=ALU.mult)
                if f["use_bqkv"]:
                    for oc in range(8):
                        nc.vector.tensor_scalar(qkc[:, oc : oc + 1],
                                                qkc[:, oc : oc + 1],
                                                W["bqkvc"][oc], None,
                                                op0=ALU.add)
                if self.dbg:
                    self.debug_dump(f"qkc{l}", [(0, qkc)])
                    self.debug_dump(f"vrow{l}", [(0, vrow)])
                    self.debug_dump(f"xs{l}",
                                    [(s, t) for (s, _), t in zip(XCH, xs)])

                def qk_emit(ch_ids):
                    for h in range(NH):
                        for nm, base in (("q", h * HD), ("k", EMB + h * HD)):
                            dst = qkt[nm, h]
                            for qsi in ch_ids:
                                qs, qsz = qch[qsi]
                                ps = qkv_ps.tile([HD, QCH], F32, name="psQK",
                                                 tag="psQK")
                                nc.tensor.matmul(
                                    ps[:, :qsz],
                                    wqk8r[:, :, base : base + HD],
                                    x8r[:, :, qs : qs + qsz], perf_mode=DRM,
                                    start=True, stop=False)
                                nc.tensor.matmul(
                                    ps[:, :qsz],
                                    W["wqk8c"][:, base : base + HD],
                                    x8c[:, qs : qs + qsz], start=False,
                                    stop=True)
                                oc = base // HD
                                nc.vector.tensor_scalar(
                                    dst[:, qs : qs + qsz], ps[:, :qsz],
                                    qkc[:, oc : oc + 1], None,
                                    op0=ALU.add)

                def v_emit(parts):
                    for j, par in parts:
                        if j not in vxt:
                            vxt[j] = apool.tile([128, 2 * NH * HD2], F8,
                                                name=f"vx{j}", tag=f"vx{j}")
                        vt4 = vxt[j].rearrange("p (i h d) -> p i h d", i=2,
                                               h=NH)
                        ts = (2 * j + par) * 128
                        ps = qkv_ps.tile([128, EMB], F32, name="psV",
                                         tag="psV")
                        nc.tensor.matmul(
                            ps, x8r[:, :, ts : ts + 128], wv8r[:, :, :EMB],
                            perf_mode=DRM, start=True, stop=False)
                        nc.tensor.matmul(
                            ps, x8c[:, ts : ts + 128], W["wv8c"],
                            start=False, stop=False)
                        if f["use_bqkv"]:
                            nc.tensor.matmul(ps, self._ones_row[:, :128],
                                             W["bqv"], start=False,
                                             stop=False)
                        nc.tensor.matmul(ps, self._ones_row[:, :128], vrow,
                                         start=False, stop=True)
                        nc.vector.tensor_copy(
                            vt4[:, par, :, :HD],
                            ps.rearrange("p (h d) -> p h d", h=NH))
                        nc.gpsimd.memset(vt4[:, par, :, HD : HD + 1], 1.0)
                        nc.gpsimd.memset(vt4[:, par, :, HD + 1 : HD + 2], 0.0)

                allp = [(j, par) for j in range(npair) for par in range(2)]
                p1 = [(j, par) for j, par in allp
                      if (2 * j + par) * 128 + 128 <= QH]
                p2 = [jp for jp in allp if jp not in p1]
                h1_ids = [i for i, (qs, qsz) in enumerate(qch)
                          if qs + qsz <= QH]
                h2_ids = [i for i in range(len(qch)) if i not in h1_ids]
                qk_emit(h1_ids)
                v_emit(p1)
                if pending is not None:
                    pending(qkv_ps)
                qk_emit(h2_ids)
                v_emit(p2)

            vx = [vxt[j] for j in range(npair)]
            self._osbB = {}
            self._osb8 = {}
            self._den = {}

            def attention_qh(att_ps, qhi, qhs, qhsz):
                den_d = self.dscr.tile([NH, 1, qhsz], BF16,
                                       name=f"den{l}_{qhi}")
                self._den[qhi] = den_d
                osbs = []
                for h in range(NH):
                    pso = att_ps.tile([HD2, QH], F32, name="pso",
                                      tag="pso", bufs=1)
                    for j in range(npair):
                        et = exp_pool.tile([128, 2 * QH], F8, name="et",
                                           tag="et", bufs=2)
                        etr = et.rearrange("p (i t) -> p i t", i=2)
                        for par in range(2):
                            kts = (2 * j + par) * 128
                            pss = att_ps.tile([128, QH], F32, name="pss",
                                              tag="pss", bufs=2)
                            for ss, ssz in chunks(qhsz, PCH):
                                nc.tensor.matmul(
                                    pss[:, ss : ss + ssz],
                                    qkt["k", h][:, kts : kts + 128],
                                    qkt["q", h][:,
                                                qhs + ss : qhs + ss + ssz],
                                    start=True, stop=True)
                            nc.scalar.activation(
                                etr[:, par, :qhsz], pss[:, :qhsz], AF.Exp,
                                bias=self._expb, scale=scale)
                        for ss, ssz in chunks(qhsz, PCH):
                            nc.tensor.matmul(
                                pso[:, ss : ss + ssz],
                                vx[j].rearrange(
                                    "p (i x) -> p i x", i=2)[
                                    :, :, h * HD2 : (h + 1) * HD2],
                                etr[:, :, ss : ss + ssz],
                                perf_mode=DRM,
                                start=(j == 0), stop=(j == npair - 1))
                    ob = apool.tile([HD1, QH], BF16, name=f"osbB{h}",
                                    tag=f"osbB{h}", bufs=1)
                    nc.vector.tensor_copy(ob[:, :qhsz], pso[:HD1, :qhsz])
                    nc.sync.dma_start(out=den_d[h],
                                      in_=ob[HD : HD + 1, :qhsz])
                    osbs.append(ob)
                self._osbB[qhi] = osbs

            def post_den(qhi, qhs, qhsz):
                # denominator -> reciprocal roundtrip; per-head normalize
                # fused into the fp8 quantize of the attention numerator
                osbs = self._osbB[qhi]
                den_d = self._den[qhi]
                nwq = qhsz // 16
                d64 = rbpool.tile([64, nwq], BF16, name="d64", tag="d64")
                nc.sync.dma_start(
                    out=d64,
                    in_=den_d.rearrange("h o (p w) -> (h o p) w", p=16))
                df = rbpool.tile([64, nwq], F32R, name="df", tag="df")
                nc.vector.tensor_scalar(df, d64, self.padc, 1.0 / OSCL,
                                        op0=ALU.subtract, op1=ALU.mult)
                r64 = rbpool.tile([64, nwq], BF16, name="r64", tag="r64")
                nc.vector.reciprocal(r64, df)
                rec_d = self.dscr.tile([NH, 16, nwq], BF16,
                                       name=f"rec{l}_{qhi}")
                nc.sync.dma_start(
                    out=rec_d.rearrange("h p w -> (h p) w"), in_=r64)
                o8 = []
                for h in range(NH):
                    t = apool.tile([HD, QH], BF16, name=f"osbN{h}",
                                   tag=f"osbN{h}", bufs=2)
                    o8.append(t)
                self._osb8[qhi] = o8
                for h in range(NH):
                    rb = rbpool.tile([HD, QH], BF16, name=f"rb{h}",
                                     tag=f"rb{h & 1}")
                    nc.sync.dma_start(
                        out=rb[:, :qhsz],
                        in_=rec_d[h].rearrange(
                            "p w -> (p w)").partition_broadcast(HD))
                    nc.vector.tensor_tensor(
                        o8[h][:, :qhsz],
                        osbs[h][:HD, :qhsz], rb[:, :qhsz], op=ALU.mult)

            def post_proj(qhi, qhs, qhsz, ps_pool):
                # out-proj (head-pair DoubleRow) + descale + residual,
                # then LN1 stats (apply is deferred to post_b)
                o8 = self._osb8[qhi]
                for qs0, qsz in chunks(qhsz, QCH):
                    qs = qhs + qs0
                    for mi, (ms, msz) in enumerate(XCH):
                        ps = ps_pool.tile([128, QCH], F32, name="psPJ",
                                          tag="aux", bufs=2)
                        for h in range(NH):
                            nc.tensor.matmul(
                                ps[:msz, :qsz],
                                W["wob"][h][:, ms : ms + msz],
                                o8[h][:, qs0 : qs0 + qsz],
                                start=(h == 0), stop=(h == NH - 1))
                        nc.vector.scalar_tensor_tensor(
                            z[mi][:, qs : qs + qsz], ps[:msz, :qsz],
                            1.0 / OSCL, xt[mi][:, qs : qs + qsz],
                            op0=ALU.mult, op1=ALU.add)
                        if f["use_bo"]:
                            nc.vector.tensor_scalar(
                                z[mi][:, qs : qs + qsz],
                                z[mi][:, qs : qs + qsz],
                                W["boc"][mi], None, op0=ALU.add)
                return self.emit_ln_stats(f"ln1_{l}_{qhi}", z, qhs, qhsz,
                                          ps_pool, sq_pool, lnp)

            def post_b(qhi, qhs, qhsz, ab2, ps_pool):
                self.emit_ln_apply(z, y, qhs, qhsz, ps_pool, ab2,
                                   W.get("ln1s"), W.get("ln1b"),
                                   f["ln1_trivial"][l])
                for mi in range(2):
                    nc.vector.tensor_copy(y8r[:, mi, qhs : qhs + qhsz],
                                          y[mi][:, qhs : qhs + qhsz])
                nc.vector.tensor_copy(y8c[:, qhs : qhs + qhsz],
                                      y[2][:, qhs : qhs + qhsz])

            fc = cpool.tile([128, 16], F32, name="fc", tag="fc")
            with ExitStack() as aes:
                att_ps = aes.enter_context(
                    tc.tile_pool(name=f"attps{l}", bufs=1, space="PSUM"))
                attention_qh(att_ps, 0, *qhch[0])
                post_den(0, *qhch[0])
                attention_qh(att_ps, 1, *qhch[1])
                ab0 = post_proj(0, *qhch[0], att_ps)
                post_b(0, *qhch[0], ab0, att_ps)
                # FFN1 correction columns dW1^T @ mean(y) -> gelu bias
                ys = []
                for mi, (ms, msz) in enumerate(XCH):
                    c = cpool.tile([msz, 1], BF16, name=f"ys{mi}",
                                   tag=f"xs{mi}")
                    nc.vector.reduce_sum(c, y[mi][:, :SMPL], axis=AX.X)
                    ys.append(c)
                for fi in range(16):
                    psc = att_ps.tile([128, 1], F32, name="psfc", tag="aux",
                                      bufs=2)
                    fs = fi * 128
                    for ki in range(3):
                        nc.tensor.matmul(psc,
                                         W["dwf1"][ki][:, fs : fs + 128],
                                         ys[ki], start=(ki == 0),
                                         stop=(ki == 2))
                    nc.vector.tensor_scalar(fc[:, fi : fi + 1], psc,
                                            CSC, None, op0=ALU.mult)
                    if f["use_bf1"]:
                        nc.vector.tensor_scalar(fc[:, fi : fi + 1],
                                                fc[:, fi : fi + 1],
                                                W["bf1c"][fi], None,
                                                op0=ALU.add)
                if self.dbg:
                    self.debug_dump(f"fc{l}", [(0, fc)])
                    self.debug_dump(f"ysum{l}",
                                    [(s, t) for (s, _), t in zip(XCH, ys)])
                post_den(1, *qhch[1])

            # ---- FFN; qh1's attention epilogue + LN1 interleave under it
            z2 = [xpool.tile([msz, Tpad], F32R, name=f"z2_{l}_{mi}",
                             tag=f"xt{mi}") for mi, (ms, msz) in enumerate(XCH)]
            xnew = z2  # LN2 applies in place
            x8n = self.x8pool.tile([128, 2 * Tpad], F8, name=f"x8n_{l}",
                                   tag="x8")
            x8nc = self.x8pool.tile([104, Tpad], F8, name=f"x8nc_{l}",
                                    tag="x8c")
            x8nr = x8n.rearrange("p (i t) -> p i t", i=2)
            wf18r = W["wf18"].rearrange("p (i m) -> p i m", i=2)

            with ExitStack() as es:
                f1_ps = es.enter_context(
                    tc.tile_pool(name=f"f1ps{l}", bufs=2, space="PSUM"))
                f2_ps = es.enter_context(
                    tc.tile_pool(name=f"f2ps{l}", bufs=2, space="PSUM"))
                ln2_ps = es.enter_context(
                    tc.tile_pool(name=f"ln2ps{l}", bufs=2, space="PSUM"))
                hpool = es.enter_context(tc.tile_pool(name=f"hp{l}", bufs=2))
                sq2_pool = es.enter_context(tc.tile_pool(name=f"sq2{l}",
                                                         bufs=1))
                lnp2 = es.enter_context(tc.tile_pool(name=f"ln2p{l}",
                                                     bufs=1))

                def ffn1_qh(qhs, qhsz):
                    h8 = [hpool.tile([128, 2 * QH], F8, name=f"h8_{jp}",
                                     tag=f"h8_{jp}", bufs=1)
                          for jp in range(8)]
                    for fi in range(16):
                        fs = fi * 128
                        ps = f1_ps.tile([128, QH], F32, name="psF1",
                                        tag="psF1", bufs=2)
                        for ss, ssz in chunks(qhsz, PCH):
                            nc.tensor.matmul(
                                ps[:, ss : ss + ssz],
                                wf18r[:, :, fs : fs + 128],
                                y8r[:, :, qhs + ss : qhs + ss + ssz],
                                perf_mode=DRM, start=True, stop=False)
                            nc.tensor.matmul(
                                ps[:, ss : ss + ssz],
                                W["wf18c"][:, fs : fs + 128],
                                y8c[:, qhs + ss : qhs + ss + ssz],
                                start=False, stop=True)
                        nc.scalar.activation(
                            h8[fi // 2].rearrange(
                                "p (i t) -> p i t", i=2)[:, fi % 2, :qhsz],
                            ps[:, :qhsz], AF.Gelu, bias=fc[:, fi : fi + 1])
                    return h8

                def ffn2_qh(h8, qhs, qhsz):
                    for qs0, qsz in chunks(qhsz, QCH):
                        qs = qhs + qs0
                        for mi, (ms, msz) in enumerate(XCH):
                            ps2 = f2_ps.tile([128, QCH], F32, name="psF2",
                                             tag="psF2", bufs=2)
                            for jp in range(8):
                                nc.tensor.matmul(
                                    ps2[:msz, :qsz],
                                    W["wf28"][jp].rearrange(
                                        "p (i m) -> p i m",
                                        i=2)[:, :, ms : ms + msz],
                                    h8[jp].rearrange(
                                        "p (i t) -> p i t",
                                        i=2)[:, :, qs0 : qs0 + qsz],
                                    perf_mode=DRM, start=(jp == 0),
                                    stop=(jp == 7))
                            nc.vector.scalar_tensor_tensor(
                                z2[mi][:, qs : qs + qsz], ps2[:msz, :qsz],
                                mc[:msz, mi : mi + 1],
                                y[mi][:, qs : qs + qsz], op0=ALU.add,
                                op1=ALU.add)
                            if f["use_bf2"]:
                                nc.vector.tensor_scalar(
                                    z2[mi][:, qs : qs + qsz],
                                    z2[mi][:, qs : qs + qsz],
                                    W["bf2c"][mi], None, op0=ALU.add)

                h8a = ffn1_qh(*qhch[0])
                ab1 = post_proj(1, *qhch[1], ln2_ps)
                # FFN2 correction columns dW2^T @ mean(h) -> residual bias
                hs = cpool.tile([128, 16], BF16, name="hs", tag="hs")
                for fi in range(16):
                    nc.vector.reduce_sum(
                        hs[:, fi : fi + 1],
                        h8a[fi // 2][:, (fi % 2) * QH : (fi % 2) * QH + SMPL],
                        axis=AX.X)
                mc = cpool.tile([128, 3], F32R, name="mc", tag="mc")
                if self.dbg:
                    nc.vector.memset(mc, 0.0)
                for mi, (ms, msz) in enumerate(XCH):
                    psc = ln2_ps.tile([128, 1], F32, name="psmc", tag="aux",
                                      bufs=2)
                    for kc in range(16):
                        nc.tensor.matmul(psc[:msz],
                                         W["dwf2"][kc][:, ms : ms + msz],
                                         hs[:, kc : kc + 1],
                                         start=(kc == 0), stop=(kc == 15))
                    nc.vector.tensor_scalar(mc[:msz, mi : mi + 1], psc[:msz],
                                            CSC, None, op0=ALU.mult)
                if self.dbg:
                    self.debug_dump(f"mcd{l}", [(0, mc)])
                    self.debug_dump(f"hsd{l}", [(0, hs)])
                ffn2_qh(h8a, *qhch[0])
                post_b(1, *qhch[1], ab1, ln2_ps)
                h8b = ffn1_qh(*qhch[1])
                st0 = self.emit_ln_stats(f"ln2_{l}_0", z2, *qhch[0],
                                         ln2_ps, sq2_pool, lnp2)
                ffn2_qh(h8b, *qhch[1])
                self.emit_ln_apply(z2, xnew, *qhch[0], ln2_ps, st0,
                                   W.get("ln2s"), W.get("ln2b"),
                                   f["ln2_trivial"][l])
                self.emit_x8(xnew, x8nr, x8nc, *qhch[0])
                st1 = self.emit_ln_stats(f"ln2_{l}_1", z2, *qhch[1],
                                         ln2_ps, sq2_pool, lnp2)
                if l + 1 < DEPTH:
                    warm = hpool.tile([1, 1], F32, name="warm", tag="warm")
                    nc.scalar.activation(warm, self._ones_row[:, :1], AF.Exp)

                qhs1, qhsz1 = qhch[1]
                ln2s, ln2b = W.get("ln2s"), W.get("ln2b")
                triv2 = f["ln2_trivial"][l]

                def pending_new(ps_pool):
                    self.emit_ln_apply(z2, xnew, qhs1, qhsz1, ps_pool, st1,
                                       ln2s, ln2b, triv2)
                    self.emit_x8(xnew, x8nr, x8nc, qhs1, qhsz1)

                if self.dbg:
                    # dbg runs apply immediately so the dumps see final data
                    pending_new(ln2_ps)
                    pending_new = None

            if self.dbg:
                self.debug_dump(f"y{l}", [(s, t) for (s, _), t in zip(XCH, y)])
                self.debug_dump(f"x{l + 1}",
                                [(s, t) for (s, _), t in zip(XCH, xnew)])
            return xnew, x8n, x8nc, pending_new

    def emit_x8(self, xnew, x8nr, x8nc, qhs, qhsz):
        # DVE, not Pool: these sit on the serial LN-apply -> qkv/FFN
        # chain; same-ring issue avoids cross-engine semaphore hops
        nc = self.nc
        for mi in range(2):
            nc.vector.tensor_copy(x8nr[:, mi, qhs : qhs + qhsz],
                                  xnew[mi][:, qhs : qhs + qhsz])
        nc.vector.tensor_copy(x8nc[:, qhs : qhs + qhsz],
                              xnew[2][:, qhs : qhs + qhsz])

    # ---------- layernorm over partition (EMB) axis, one q-half ----------
    def emit_ln_stats(self, name, z, qhs, qhsz, ps_pool, sq_pool, lnp,
                      row_bufs=1):
        nc = self.nc
        inv_d = 1.0 / EMB
        ones_col = self._ones_col
        i32 = mybir.dt.int32
        sum_t = lnp.tile([1, QH], F32, name="sum_t", tag="sum_t",
                         bufs=row_bufs)
        sq_t = lnp.tile([1, QH], F32, name="sq_t", tag="sq_t", bufs=row_bufs)
        a2 = self.lnrows.tile([1, QH], F32R, name="a2", tag="a2", bufs=2)
        b2 = self.lnrows.tile([1, QH], F32R, name="b2", tag="b2", bufs=2)

        for qs0, qsz in chunks(qhsz, QCH):
            qs = qhs + qs0
            psm = ps_pool.tile([1, QCH], F32, name="psm", tag="aux", bufs=2)
            pssq = ps_pool.tile([1, QCH], F32, name="pssq", tag="aux", bufs=2)
            for mi, (ms, msz) in enumerate(XCH):
                sq = sq_pool.tile([msz, QCH], BF16, name="sq", tag=f"sq{mi}")
                nc.gpsimd.tensor_tensor(sq[:, :qsz], z[mi][:, qs : qs + qsz],
                                        z[mi][:, qs : qs + qsz], op=ALU.mult)
                nc.tensor.matmul(psm[:, :qsz], ones_col[:msz, :],
                                 z[mi][:, qs : qs + qsz], start=(mi == 0),
                                 stop=(mi == len(XCH) - 1))
                nc.tensor.matmul(pssq[:, :qsz], self._ones_col_b[:msz, :],
                                 sq[:, :qsz], start=(mi == 0),
                                 stop=(mi == len(XCH) - 1))
            nc.vector.tensor_copy(sum_t[:, qs0 : qs0 + qsz], psm[:, :qsz])
            nc.vector.tensor_copy(sq_t[:, qs0 : qs0 + qsz], pssq[:, :qsz])

        # rows -> 32 partitions via DRAM
        nw = qhsz // 32
        st_d = self.dscr.tile([2, 1, qhsz], F32, name=f"{name}_std")
        nc.sync.dma_start(out=st_d[0], in_=sum_t[:, :qhsz])
        nc.sync.dma_start(out=st_d[1], in_=sq_t[:, :qhsz])
        st32 = lnp.tile([32, 2 * nw], F32, name="st32", tag="st32")
        nc.sync.dma_start(
            out=st32.rearrange("p (i w) -> p i w", i=2),
            in_=st_d.rearrange("i o (p w) -> p i (o w)", p=32))

        mean = lnp.tile([32, nw], F32, name="mean", tag="mean")
        nc.vector.tensor_scalar(mean, st32[:, 0:nw], inv_d, None,
                                op0=ALU.mult)
        v0 = lnp.tile([32, nw], F32, name="v0", tag="v0")
        nc.vector.tensor_scalar(v0, st32[:, nw : 2 * nw], inv_d, EPS,
                                op0=ALU.mult, op1=ALU.add)
        m2 = lnp.tile([32, nw], F32, name="m2", tag="m2")
        nc.vector.tensor_tensor(m2, mean, mean, op=ALU.mult)
        var = lnp.tile([32, nw], F32, name="var", tag="var")
        nc.vector.tensor_tensor(var, v0, m2, op=ALU.subtract)

        # rsqrt via bit-trick seed + 2 Newton iterations (DVE only)
        seed = lnp.tile([32, nw], i32, name="seed", tag="seed")
        nc.vector.tensor_scalar(seed, var.bitcast(i32), 1, None,
                                op0=ALU.logical_shift_right)
        magic = lnp.tile([32, nw], i32, name="magic", tag="magic")
        nc.vector.memset(magic, 0x5F3759DF)
        y0 = lnp.tile([32, nw], i32, name="y0", tag="y0")
        nc.vector.tensor_tensor(y0, magic, seed, op=ALU.subtract)
        yv = y0.bitcast(F32)
        t1 = lnp.tile([32, nw], F32, name="t1", tag="t1")
        ab = lnp.tile([32, 2 * nw], F32R, name="ab", tag="ab")
        for it in range(2):
            nc.vector.tensor_tensor(t1, var, yv, op=ALU.mult)
            nc.vector.tensor_tensor(t1, t1, yv, op=ALU.mult)
            nc.vector.tensor_scalar(t1, t1, -0.5, 1.5, op0=ALU.mult,
                                    op1=ALU.add)
            if it == 0:
                nc.vector.tensor_tensor(yv, yv, t1, op=ALU.mult)
            else:
                nc.vector.tensor_tensor(ab[:, 0:nw], yv, t1, op=ALU.mult)
        nc.vector.tensor_tensor(ab[:, nw : 2 * nw], mean, ab[:, 0:nw],
                                op=ALU.mult)
        nc.vector.tensor_scalar(ab[:, nw : 2 * nw], ab[:, nw : 2 * nw],
                                -1.0, None, op0=ALU.mult)

        ab_d = self.dscr.tile([32, 2, nw], F32R, name=f"{name}_abd")
        nc.sync.dma_start(out=ab_d, in_=ab.rearrange("p (i w) -> p i w", i=2))
        for i, t in enumerate((a2, b2)):
            nc.sync.dma_start(
                out=t[:, :qhsz].rearrange("o (p w) -> o p w", p=32),
                in_=ab_d[:, i : i + 1, :].rearrange("p i w -> i p w"))

        return (a2, b2)

    def emit_ln_apply(self, z, y, qhs, qhsz, ps_pool, ab2, sc, bc,
                      trivial):
        nc = self.nc
        ones_row = self._ones_row
        a2, b2 = ab2
        for qs0, qsz in chunks(qhsz, QCH):
            qs = qhs + qs0
            psa = ps_pool.tile([128, QCH], F32, name="psa", tag="aux",
                               bufs=2)
            psb = ps_pool.tile([128, QCH], F32, name="psb", tag="aux",
                               bufs=2)
            nc.tensor.matmul(psa[:, :qsz], ones_row[:, :128],
                             a2[:, qs0 : qs0 + qsz], start=True, stop=True)
            nc.tensor.matmul(psb[:, :qsz], ones_row[:, :128],
                             b2[:, qs0 : qs0 + qsz], start=True, stop=True)
            for mi, (ms, msz) in enumerate(XCH):
                nc.vector.tensor_tensor(y[mi][:, qs : qs + qsz],
                                        z[mi][:, qs : qs + qsz],
                                        psa[:msz, :qsz], op=ALU.mult)
                nc.vector.tensor_tensor(y[mi][:, qs : qs + qsz],
                                        y[mi][:, qs : qs + qsz],
                                        psb[:msz, :qsz], op=ALU.add)
                if not trivial:
                    nc.vector.tensor_scalar(y[mi][:, qs : qs + qsz],
                                            y[mi][:, qs : qs + qsz],
                                            sc[mi], bc[mi], op0=ALU.mult,
                                            op1=ALU.add)
                if self.Tpad > self.T and qs + qsz > self.T:
                    # keep zero-padded tokens exactly zero so the pad-key
                    # denominator correction stays exact in later layers
                    nc.vector.memset(
                        y[mi][:, max(qs, self.T) : qs + qsz].bitcast(F32),
                        0.0)

    # ---------- head ----------
    def phase_head(self, xt, pending=None):
        nc, tc = self.nc, self.tc
        T = self.T
        H = self.HW

        outd = self.dout("out", (NCLS, 1))

        with ExitStack() as es:
            hpool = es.enter_context(tc.tile_pool(name="head", bufs=1))
            hps = es.enter_context(
                tc.tile_pool(name="head_ps", bufs=2, space="PSUM"))

            if pending is not None:
                pending(hps)

            hmean = []
            for mi, (ms, msz) in enumerate(XCH):
                hm = hpool.tile([msz, 1], F32, name=f"hm{mi}", tag=f"hm{mi}")
                nc.vector.reduce_sum(hm, xt[mi][:, :T], axis=AX.X)
                nc.vector.tensor_scalar(hm, hm, 1.0 / T, None, op0=ALU.mult)
                hmean.append(hm)
            if self.dbg:
                self.debug_dump("hmean",
                                [(s, t) for (s, _), t in zip(XCH, hmean)])

            h1 = []
            for mi, (ms, msz) in enumerate(chunks(C1, 128)):
                ps = hps.tile([128, 1], F32, name=f"psH1_{mi}", tag="psH")
                for ki in range(len(XCH)):
                    nc.tensor.matmul(ps[:msz], H["cw1"][ki][:, ms : ms + msz],
                                     hmean[ki], start=(ki == 0),
                                     stop=(ki == len(XCH) - 1))
                ht = hpool.tile([msz, 1], F32, name=f"h1_{mi}", tag=f"h1_{mi}")
                nc.scalar.activation(ht, ps[:msz], AF.Gelu, bias=H["cb1"][mi])
                h1.append(ht)

            ps = hps.tile([128, 1], F32, name="psH2", tag="psH")
            for ki in range(len(H["cw2"])):
                nc.tensor.matmul(ps[:C2], H["cw2"][ki], h1[ki],
                                 start=(ki == 0),
                                 stop=(ki == len(H["cw2"]) - 1))
            h2 = hpool.tile([C2, 1], F32)
            nc.scalar.activation(h2, ps[:C2], AF.Relu, bias=H["cb2"][0])

            ps3 = hps.tile([128, 1], F32, name="psH3", tag="psH")
            nc.tensor.matmul(ps3[:NCLS], H["cw3"][0], h2, start=True,
                             stop=True)
            res = hpool.tile([NCLS, 1], F32)
            nc.scalar.activation(res, ps3[:NCLS], AF.Identity,
                                 bias=H["cb3"][0])
            nc.sync.dma_start(out=outd, in_=res)


# ---------------------------------------------------------------------------
# Host side
# ---------------------------------------------------------------------------

def _build_counts(C):
    """S[b, r, n] = #{v in win(n): C[b, v] == r} for r in 0..NROI."""
    Bn = C.shape[0]
    S = np.zeros((Bn, NROI + 1, NB), np.int32)
    b_idx = np.arange(Bn)[:, None]
    n_idx = np.arange(NB)[None, :]
    for di in range(KS):
        for dj in range(KS):
            for dk in range(KS):
                sub = C[:, di : di + 2 * (NBLK - 1) + 1 : ST,
                        dj : dj + 2 * (NBLK - 1) + 1 : ST,
                        dk : dk + 2 * (NBLK - 1) + 1 : ST].reshape(Bn, NB)
                np.add.at(S, (b_idx, sub, n_idx), 1)
    return S


def _f8(x):
    return np.ascontiguousarray(np.asarray(x, np.float32).astype(NPF8))


def _bf(x):
    return np.ascontiguousarray(
        np.asarray(x, np.float32).astype(ml_dtypes.bfloat16))


def _pack_dr(w, k0, ksz=256):
    """Pack rows [k0, k0+256) of w into DoubleRow layout [128, 2*M] fp8."""
    M = w.shape[1]
    out = np.empty((128, 2, M), np.float32)
    out[:, 0, :] = w[k0 : k0 + 128]
    out[:, 1, :] = w[k0 + 128 : k0 + 256]
    return _f8(out.reshape(128, 2 * M))


def host_prepare(inputs):
    inp = {k: np.asarray(v) for k, v in inputs.items()}
    F_roi = inp["F_roi"].astype(np.float32)
    C = inp["C"].astype(np.int64)

    S = _build_counts(C)
    valid = S[:, 1:, :].sum(axis=(0, 1)) > 0
    vidx = np.nonzero(valid)[0]
    T = int(len(vidx))
    Tpad = ((T + 255) // 256) * 256
    s_t = np.zeros((C.shape[0], NROI, Tpad), np.float32)
    s_t[:, :, :T] = S[:, 1:, :][:, :, vidx].astype(np.float32)

    f32 = lambda x: np.ascontiguousarray(np.asarray(x), dtype=np.float32)
    col = lambda x: f32(x).reshape(-1, 1)
    row = lambda x: f32(x).reshape(1, -1)

    shared = {
        "w1": f32(inp["ffn_w1"]), "b1c": col(inp["ffn_b1"]),
        "w2": f32(inp["ffn_w2"]), "b2r": row(inp["ffn_b2"]),
        "cw1": f32(inp["cw1"]), "cb1c": col(inp["cb1"]),
        "cw2": f32(inp["cw2"]), "cb2c": col(inp["cb2"]),
        "cw3": f32(inp["cw3"]), "cb3c": col(inp["cb3"]),
        "ones_col": np.ones((128, 1), np.float32),
        "ones_row": np.ones((1, 128), np.float32),
    }
    for l in range(DEPTH):
        wqkv = f32(inp["wqkv"][l])            # (360, 1080)
        pk = np.empty((128, 2, 3 * EMB), np.float32)
        pk[:, 0] = wqkv[0:128]
        pk[:, 1] = wqkv[128:256]
        shared[f"wqk8d{l}"] = _f8(pk[:, :, : 2 * EMB].reshape(128, -1))
        pkv = np.zeros((128, 2, EMBP), np.float32)
        pkv[:, :, :EMB] = pk[:, :, 2 * EMB :]
        shared[f"wv8d{l}"] = _f8(pkv.reshape(128, -1))
        shared[f"wqk8c{l}"] = _f8(wqkv[256:, : 2 * EMB])
        shared[f"wv8c{l}"] = _f8(wqkv[256:, 2 * EMB :])
        wo = f32(inp["wo"][l])                # (360, 360)
        shared[f"wob{l}"] = _bf(wo.reshape(NH, HD, EMB))
        wf1 = f32(inp["wf1"][l])              # (360, 2048)
        shared[f"wf18d{l}"] = _pack_dr(wf1, 0)
        shared[f"wf18c{l}"] = _f8(wf1[256:])
        wf2 = f32(inp["wf2"][l])              # (2048, 360)
        w28 = np.zeros((8, 128, 2, EMBP), np.float32)
        for j in range(8):
            w28[j, :, 0, :EMB] = wf2[256 * j : 256 * j + 128]
            w28[j, :, 1, :EMB] = wf2[256 * j + 128 : 256 * j + 256]
        shared[f"wf28d{l}"] = _f8(w28.reshape(8, 128, 2 * EMBP))
        # fp8-weight residuals for the rank-1 token-mean correction,
        # scaled UP by DSC so the tiny residuals stay out of fp8's
        # flush-to-zero range; the device folds 1/(DSC*SMPL) back in
        d8 = lambda w: _f8((w - _f8(w).astype(np.float32)) * DSC)
        shared[f"dwqk{l}"] = d8(wqkv[:, : 2 * EMB])
        shared[f"dwv{l}"] = d8(wqkv[:, 2 * EMB :])
        shared[f"dwf1_{l}"] = d8(wf1)
        shared[f"dwf2_{l}"] = d8(wf2)

        shared[f"bqkv{l}c"] = col(inp["bqkv"][l][: 2 * EMB])
        shared[f"bqkv{l}vr"] = row(inp["bqkv"][l][2 * EMB :])
        shared[f"bo{l}c"] = col(inp["bo"][l])
        shared[f"ln1s{l}c"] = col(inp["ln1_s"][l])
        shared[f"ln1b{l}c"] = col(inp["ln1_b"][l])
        shared[f"bf1_{l}c"] = col(inp["bf1"][l])
        shared[f"bf2_{l}c"] = col(inp["bf2"][l])
        shared[f"ln2s{l}c"] = col(inp["ln2_s"][l])
        shared[f"ln2b{l}c"] = col(inp["ln2_b"][l])

    flags = {
        "use_b2": bool(np.any(np.asarray(inp["ffn_b2"]) != 0)),
        "use_bqkv": bool(np.any(np.asarray(inp["bqkv"]) != 0)),
        "use_bo": bool(np.any(np.asarray(inp["bo"]) != 0)),
        "use_bf1": bool(np.any(np.asarray(inp["bf1"]) != 0)),
        "use_bf2": bool(np.any(np.asarray(inp["bf2"]) != 0)),
        "ln1_trivial": [bool(np.all(np.asarray(inp["ln1_s"][l]) == 1)
                             and np.all(np.asarray(inp["ln1_b"][l]) == 0))
                        for l in range(DEPTH)],
        "ln2_trivial": [bool(np.all(np.asarray(inp["ln2_s"][l]) == 1)
                             and np.all(np.asarray(inp["ln2_b"][l]) == 0))
                        for l in range(DEPTH)],
    }

    in_maps = []
    for b in range(F_roi.shape[0]):
        m = dict(shared)
        m["f_roiT"] = np.ascontiguousarray(F_roi[b].T)
        m["s_t"] = np.ascontiguousarray(s_t[b])
        in_maps.append(m)
    return in_maps, T, Tpad, flags


def build_program(T, Tpad, flags, dbg=False):
    nc = bacc.Bacc("TRN2", target_bir_lowering=False, debug=False,
                   enable_asserts=False, num_devices=B)
    with tile.TileContext(nc) as tc:
        with nc.allow_low_precision("fp8/bf16 matmul operand plumbing"):
            with ExitStack() as ctx:
                bld = Builder(nc, tc, ctx, T, Tpad, flags, dbg=dbg)
                bld.build()
    nc.compile()
    return nc


def kernel(**inputs):
    in_maps, T, Tpad, flags = host_prepare(inputs)
    nc = build_program(T, Tpad, flags)
    res = run_bass_kernel_spmd(nc, in_maps, core_ids=list(range(len(in_maps))))
    out = np.stack([r["out"].reshape(NCLS) for r in res.results])
    return out.astype(np.float32)
